# revision 1
# baseline (speedup 1.0000x reference)
"""ContentGuidedAttention Trainium2 kernel.

Full NxN single-head cross-attention + out-proj + residual + LayerNorm,
for B=4, C=256, H=W=64 (N=4096 tokens), distributed over 8 NeuronCores:
core i handles batch i//2, query-half i%2 (2048 queries, all 4096 keys).
No collectives: K/V are computed redundantly on the two cores sharing a
batch (~5% extra FLOPs).

Layout strategy (all channel-major, zero transposes):
  - Q^T, K^T computed as [C, n] (channels on partitions) in bf16
  - V computed token-major [n, C] in bf16
  - S^T = K Q^T computed as [k, q] psum tiles; exp on ACT -> P^T bf16
  - softmax denominator: contiguous DVE chunk-tree then a ones-vector
    matmul reduces the 128 partitions -> [1, q]
  - reciprocals and rsqrt run on ACT as exp(-ln x) / exp(-0.5 ln x):
    Ln and Exp share one activation-table set, so no table switches
  - row -> all-partition replication via K=1 ones-column matmuls
  - PV: O^T[c, q] = sum_k V[k,c] P^T[k,q]; out-proj keeps channel-major
  - LN entirely per-query-block, overlapped with the next block's
    attention; affine via ACT Identity with per-partition scale/bias
Projection matmuls run in float32r (full PE rate at free dim >= 256).
"""

import numpy as np

import concourse.bass as bass
import concourse.mybir as mybir
import concourse.tile as tile
from concourse import bacc
from concourse.bass import ds, ts
from concourse.bass_utils import run_bass_kernel_spmd

F32 = mybir.dt.float32
F32R = mybir.dt.float32r
BF16 = mybir.dt.bfloat16
AF = mybir.ActivationFunctionType
OP = mybir.AluOpType

B = 4
C = 256
N = 4096          # tokens per batch
NQ = 2048         # queries per core
QB = 512          # query block
NQB = NQ // QB    # 4
NKC = N // 128    # 32 key chunks
NKR = 4           # key ranges (1024 keys each) for K^T / V tiles
SCALE = (C // 8) ** -0.5
LN_EPS = 1e-5

_CACHE = {}


def _build_nc(dbg=False):
    nc = bacc.Bacc("TRN2", target_bir_lowering=False, debug=False)

    low_d = nc.declare_dram_parameter("low", [C, NQ], F32R, isOutput=False)
    high_d = nc.declare_dram_parameter("high", [C, N], F32R, isOutput=False)
    # weights are passed pre-transposed: [c_in, c_out]
    wq_d = nc.declare_dram_parameter("wq", [C, C], F32R, isOutput=False)
    wk_d = nc.declare_dram_parameter("wk", [C, C], F32R, isOutput=False)
    wv_d = nc.declare_dram_parameter("wv", [C, C], F32R, isOutput=False)
    wo_d = nc.declare_dram_parameter("wo", [C, C], F32R, isOutput=False)
    # qb, kb, ob, lng, lnb prepacked host-side as [128, 10]
    pvec_d = nc.declare_dram_parameter("pvec", [128, 10], F32, isOutput=False)
    out_d = nc.declare_dram_parameter("out", [C, NQ], F32, isOutput=True)
    dbg_d = {}
    if dbg:
        for nm, shp, dt_ in [
            ("dbg_rcp", [1, 512], F32), ("dbg_mu", [1, 512], F32),
            ("dbg_var", [1, 512], F32), ("dbg_rstd", [1, 512], F32),
            ("dbg_tT", [128, 512], BF16), ("dbg_ot", [128, 2, QB], F32),
            ("dbg_qt", [128, 2, QB], BF16), ("dbg_kt", [128, 2, 1024], BF16),
            ("dbg_v", [128, 8, C], BF16), ("dbg_pt", [128, 8, QB], BF16),
        ]:
            dbg_d[nm] = nc.declare_dram_parameter(nm, shp, dt_, isOutput=True)

    with tile.TileContext(nc) as tc:
        with (
            tc.tile_pool(name="persist", bufs=1) as pp,
            tc.tile_pool(name="high", bufs=3) as high_pool,
            tc.tile_pool(name="pt", bufs=5) as pt_pool,
            tc.tile_pool(name="ot", bufs=2) as ot_pool,
            tc.tile_pool(name="scratch", bufs=2) as scr_pool,
            tc.tile_pool(name="rowscr", bufs=1) as row_pool,
            tc.tile_pool(name="outsb", bufs=2) as out_pool,
            tc.tile_pool(name="st_ps", bufs=2, space="PSUM") as st_ps,
            tc.tile_pool(name="acc_ps", bufs=3, space="PSUM") as acc_ps,
            tc.tile_pool(name="row_ps", bufs=1, space="PSUM") as row_ps,
        ):
            # ---------------- constants / parameters ----------------
            # one tile per weight matrix ([cin_p, cin_chunk, cout]); K/V
            # weights load first so the K/V projections start ASAP
            pvec = pp.tile([128, 10], F32)
            nc.scalar.dma_start(out=pvec[:, :], in_=pvec_d[:, :])
            wk_sb = pp.tile([128, 2, C], F32R)
            wv_sb = pp.tile([128, 2, C], F32R)
            wq_sb = pp.tile([128, 2, C], F32R)
            wo_sb = pp.tile([128, 2, C], F32R)
            for t, d in [(wk_sb, wk_d), (wv_sb, wv_d), (wq_sb, wq_d),
                         (wo_sb, wo_d)]:
                for j in range(2):
                    nc.scalar.dma_start(out=t[:, j, :], in_=d[ds(j * 128, 128), :])

            # memset cannot emit float32r; stage in f32 and copy (the
            # DVE tensor_copy performs the f32 -> f32r rounding walrus wants)
            stage = pp.tile([128, 128], F32)
            ones1 = pp.tile([1, 128], F32R)      # K=1 replication lhsT
            nc.vector.memset(stage[ds(0, 1), :], 1.0)
            nc.vector.tensor_copy(ones1[:, :], stage[ds(0, 1), :])
            ones128 = pp.tile([128, 1], F32R)    # partition-reduce lhsT (f32r)
            nc.vector.memset(stage[:, 0:1], 1.0)
            nc.vector.tensor_copy(ones128[:, :], stage[:, 0:1])
            ones128b = pp.tile([128, 1], BF16)   # partition-reduce lhsT (bf16)
            nc.vector.memset(ones128b[:, :], 1.0)
            epsb = pp.tile([1, 1], F32)          # LN epsilon bias
            nc.vector.memset(epsb[:, :], LN_EPS)

            QBIAS, KBIAS, OBIAS, LNG, LNB = 0, 2, 4, 6, 8

            # ---------------- K^T / V projections ----------------
            # per 1024-key-range tiles so attention can start early
            kt_sb = [
                pp.tile([128, 2, 1024], BF16, name=f"kt{r}", tag=f"kt{r}")
                for r in range(NKR)
            ]
            v_sb = [
                pp.tile([128, 8, C], BF16, name=f"v{r}", tag=f"v{r}")
                for r in range(NKR)
            ]
            for kr in range(N // 512):
                hi = high_pool.tile([128, 2, 512], F32R)
                for j in range(2):
                    nc.sync.dma_start(
                        out=hi[:, j, :],
                        in_=high_d[ds(j * 128, 128), ds(kr * 512, 512)],
                    )
                r, h = kr // 2, kr % 2
                # K^T: out [cout, k] = sum_cin wk[cin, cout] high[cin, k]
                for c in range(2):
                    kps = st_ps.tile([128, 512], F32, tag="st")
                    for j in range(2):
                        nc.tensor.matmul(
                            out=kps[:, :],
                            lhsT=wk_sb[:, j, ds(c * 128, 128)],
                            rhs=hi[:, j, :],
                            start=(j == 0), stop=(j == 1),
                        )
                    nc.vector.tensor_scalar_add(
                        out=kt_sb[r][:, c, ds(h * 512, 512)],
                        in0=kps[:, :],
                        scalar1=pvec[:, ds(KBIAS + c, 1)],
                    )
                # V: out [k, cout] = sum_cin high[cin, k] wv[cin, cout]
                for u in range(4):
                    vps = st_ps.tile([128, C], F32, tag="st")
                    for j in range(2):
                        nc.tensor.matmul(
                            out=vps[:, :],
                            lhsT=hi[:, j, ds(u * 128, 128)],
                            rhs=wv_sb[:, j, :],
                            start=(j == 0), stop=(j == 1),
                        )
                    nc.scalar.activation(
                        out=v_sb[r][:, h * 4 + u, :], in_=vps[:, :],
                        func=AF.Copy,
                    )

            # ---------------- Q^T projection (all blocks) ----------------
            low_sb = pp.tile([128, 2, NQ], F32R)
            for j in range(2):
                nc.scalar.dma_start(
                    out=low_sb[:, j, :], in_=low_d[ds(j * 128, 128), :]
                )
            qt_all = pp.tile([128, 2, NQ], BF16)
            for qb4 in range(NQB):
                for c in range(2):
                    qps = st_ps.tile([128, QB], F32, tag="st")
                    for j in range(2):
                        nc.tensor.matmul(
                            out=qps[:, :],
                            lhsT=wq_sb[:, j, ds(c * 128, 128)],
                            rhs=low_sb[:, j, ds(qb4 * QB, QB)],
                            start=(j == 0), stop=(j == 1),
                        )
                    nc.vector.tensor_scalar_add(
                        out=qt_all[:, c, ds(qb4 * QB, QB)], in0=qps[:, :],
                        scalar1=pvec[:, ds(QBIAS + c, 1)],
                    )


            # ---------------- main loop over query blocks ----------------
            # Emission is software-pipelined: block b's scalar-chain matmuls
            # (denominator, out-proj, LN stats) are emitted inside block
            # b+1's attention so the in-order PE queue never waits on the
            # DVE/ACT softmax-denominator and LayerNorm chains.

            def attention(b):
                qsl = ds(b * QB, QB)
                quarters = [
                    pt_pool.tile([128, 8, QB], BF16, tag="ptq", name=f"ptq{g}")
                    for g in range(4)
                ]
                for si in range(16):
                    sps = st_ps.tile([128, 2, QB], F32, tag="st")
                    for u in range(2):
                        kc = si * 2 + u
                        for c in range(2):
                            nc.tensor.matmul(
                                out=sps[:, u, :],
                                lhsT=kt_sb[kc // 8][:, c, ds((kc % 8) * 128, 128)],
                                rhs=qt_all[:, c, qsl],
                                start=(c == 0), stop=(c == 1),
                            )
                    nc.scalar.activation(
                        out=quarters[si // 4][:, ds((si % 4) * 2, 2), :],
                        in_=sps[:, :, :],
                        func=AF.Exp,
                        scale=SCALE,
                    )
                return quarters

            def pv(b, quarters):
                ot = ot_pool.tile([128, 2, QB], F32R, tag="ot", name=f"ot{b}")
                for c in range(2):
                    ops = acc_ps.tile([128, QB], F32, tag="acc")
                    for kc in range(NKC):
                        nc.tensor.matmul(
                            out=ops[:, :],
                            lhsT=v_sb[kc // 8][:, kc % 8, ds(c * 128, 128)],
                            rhs=quarters[kc // 8][:, kc % 8, :],
                            start=(kc == 0), stop=(kc == NKC - 1),
                        )
                    nc.vector.tensor_copy(ot[:, c, :], ops[:, :])
                return ot

            def tree(b, quarters):
                # contiguous DVE chunk tree for the softmax denominator.
                # q0..q2 fold while exp of q3 is still streaming; q3 then
                # folds shallowly so the post-attention critical path is
                # only ~2us of DVE work.
                fl = [q[:, :, :].rearrange("p a b -> p (a b)") for q in quarters]
                nc.vector.tensor_add(out=fl[0], in0=fl[0], in1=fl[1])
                nc.vector.tensor_add(out=fl[0], in0=fl[0], in1=fl[2])
                tB = scr_pool.tile([128, 4, QB], BF16, tag="tB")
                nc.vector.tensor_add(
                    out=tB[:, :, :],
                    in0=quarters[0][:, 0:4, :], in1=quarters[0][:, 4:8, :],
                )
                tT2 = scr_pool.tile([128, 2, QB], BF16, tag="tT2")
                nc.vector.tensor_add(
                    out=tT2[:, :, :], in0=tB[:, 0:2, :], in1=tB[:, 2:4, :]
                )
                p012 = scr_pool.tile([128, QB], BF16, tag="p012")
                nc.vector.tensor_add(
                    out=p012[:, :], in0=tT2[:, 0, :], in1=tT2[:, 1, :]
                )
                a3 = scr_pool.tile([128, 4, QB], BF16, tag="tB")
                nc.vector.tensor_add(
                    out=a3[:, :, :],
                    in0=quarters[3][:, 0:4, :], in1=quarters[3][:, 4:8, :],
                )
                b3 = scr_pool.tile([128, 2, QB], BF16, tag="tT2")
                nc.vector.tensor_add(
                    out=b3[:, :, :], in0=a3[:, 0:2, :], in1=a3[:, 2:4, :]
                )
                tT = scr_pool.tile([128, QB], BF16, tag="tT", name=f"tT{b}")
                nc.vector.scalar_tensor_tensor(
                    out=tT[:, :], in0=b3[:, 0, :], scalar=0.0,
                    in1=b3[:, 1, :], op0=OP.add, op1=OP.add,
                )
                nc.vector.tensor_add(
                    out=tT[:, :], in0=tT[:, :], in1=p012[:, :]
                )
                return tT

            def denom_recip(b, tT):
                dn_ps = row_ps.tile([1, QB], F32, tag="row")
                nc.tensor.matmul(
                    out=dn_ps[:, :], lhsT=ones128b[:, :], rhs=tT[:, :],
                    start=True, stop=True,
                )
                # 1/denom = exp(-ln(denom)) on ACT (same table set as Exp)
                lnrow = row_pool.tile([1, QB], F32, tag="lnrow")
                nc.scalar.activation(
                    out=lnrow[:, :], in_=dn_ps[:, :], func=AF.Ln
                )
                rcprow = row_pool.tile([1, QB], F32, tag="rcprow",
                                       name=f"rcprow{b}")
                nc.scalar.activation(
                    out=rcprow[:, :], in_=lnrow[:, :], func=AF.Exp, scale=-1.0
                )
                rcp_rep = scr_pool.tile([128, QB], F32, tag="rcprep",
                                        name=f"rcprep{b}")
                nc.gpsimd.partition_broadcast(rcp_rep[:, :], rcprow[:, :])
                return rcprow, rcp_rep

            def outproj_y(b, ot, rcp_rep):
                qsl = ds(b * QB, QB)
                y_sb = ot_pool.tile([128, 2, QB], F32R, tag="y", name=f"y{b}")
                for c in range(2):
                    pps = acc_ps.tile([128, QB], F32, tag="acc")
                    for j in range(2):
                        nc.tensor.matmul(
                            out=pps[:, :],
                            lhsT=wo_sb[:, j, ds(c * 128, 128)],
                            rhs=ot[:, j, :],
                            start=(j == 0), stop=(j == 1),
                        )
                    ysc = scr_pool.tile([128, QB], F32, tag="scr")
                    nc.vector.tensor_mul(
                        out=ysc[:, :], in0=pps[:, :], in1=rcp_rep[:, :]
                    )
                    nc.vector.scalar_tensor_tensor(
                        out=y_sb[:, c, :],
                        in0=ysc[:, :],
                        scalar=pvec[:, ds(OBIAS + c, 1)],
                        in1=low_sb[:, c, qsl].bitcast(F32),
                        op0=OP.add, op1=OP.add,
                    )
                return y_sb

            def stats_ln(b, y_sb, rcprow):
                qsl = ds(b * QB, QB)
                sy_ps = row_ps.tile([1, QB], F32, tag="row")
                for c in range(2):
                    nc.tensor.matmul(
                        out=sy_ps[:, :],
                        lhsT=ones128[:, :],
                        rhs=y_sb[:, c, :],
                        start=(c == 0), stop=(c == 1),
                    )
                murow = row_pool.tile([1, QB], F32, tag="murow")
                nc.vector.tensor_scalar_mul(
                    out=murow[:, :], in0=sy_ps[:, :], scalar1=1.0 / C
                )
                sy2_ps = row_ps.tile([1, QB], F32, tag="row")
                for c in range(2):
                    ysq = scr_pool.tile([128, QB], F32R, tag="ysq")
                    nc.vector.tensor_mul(
                        out=ysq[:, :],
                        in0=y_sb[:, c, :].bitcast(F32),
                        in1=y_sb[:, c, :].bitcast(F32),
                    )
                    nc.tensor.matmul(
                        out=sy2_ps[:, :],
                        lhsT=ones128[:, :],
                        rhs=ysq[:, :],
                        start=(c == 0), stop=(c == 1),
                    )
                # var = E[y^2] - mu^2 ; rstd = exp(-0.5 ln(var + eps))
                varrow = row_pool.tile([1, QB], F32, tag="varrow")
                nc.vector.tensor_scalar_mul(
                    out=varrow[:, :], in0=sy2_ps[:, :], scalar1=1.0 / C
                )
                mu2row = row_pool.tile([1, QB], F32, tag="mu2row")
                nc.vector.tensor_mul(
                    out=mu2row[:, :], in0=murow[:, :], in1=murow[:, :],
                )
                nc.vector.tensor_sub(
                    out=varrow[:, :], in0=varrow[:, :], in1=mu2row[:, :]
                )
                lnv = row_pool.tile([1, QB], F32, tag="lnv")
                nc.scalar.activation(
                    out=lnv[:, :], in_=varrow[:, :], func=AF.Ln, bias=epsb[:, :]
                )
                rstdrow = row_pool.tile([1, QB], F32, tag="rstdrow")
                nc.scalar.activation(
                    out=rstdrow[:, :], in_=lnv[:, :], func=AF.Exp, scale=-0.5
                )
                if dbg_d and b == NQB - 1:
                    nc.sync.dma_start(out=dbg_d["dbg_rcp"][:, :], in_=rcprow[:, :])
                    nc.sync.dma_start(out=dbg_d["dbg_mu"][:, :], in_=murow[:, :])
                    nc.sync.dma_start(out=dbg_d["dbg_var"][:, :],
                                      in_=varrow[:, :])
                    nc.sync.dma_start(out=dbg_d["dbg_rstd"][:, :],
                                      in_=rstdrow[:, :])
                mu_rep = scr_pool.tile([128, QB], F32, tag="murep")
                nc.gpsimd.partition_broadcast(mu_rep[:, :], murow[:, :])
                rs_rep = scr_pool.tile([128, QB], F32, tag="rsrep")
                nc.gpsimd.partition_broadcast(rs_rep[:, :], rstdrow[:, :])
                for c in range(2):
                    yn = scr_pool.tile([128, QB], F32, tag="scr")
                    nc.vector.tensor_sub(
                        out=yn[:, :],
                        in0=y_sb[:, c, :].bitcast(F32),
                        in1=mu_rep[:, :],
                    )
                    nc.vector.tensor_mul(
                        out=yn[:, :], in0=yn[:, :], in1=rs_rep[:, :]
                    )
                    osb = out_pool.tile([128, QB], F32)
                    nc.vector.tensor_scalar(
                        out=osb[:, :], in0=yn[:, :],
                        scalar1=pvec[:, ds(LNG + c, 1)],
                        scalar2=pvec[:, ds(LNB + c, 1)],
                        op0=OP.mult, op1=OP.add,
                    )
                    nc.scalar.dma_start(
                        out=out_d[ds(c * 128, 128), qsl], in_=osb[:, :]
                    )

            for b in range(NQB):
                quarters = attention(b)
                ot = pv(b, quarters)
                tT = tree(b, quarters)
                rcprow, rcp_rep = denom_recip(b, tT)
                y_b = outproj_y(b, ot, rcp_rep)
                stats_ln(b, y_b, rcprow)
                if dbg_d and b == NQB - 1:
                    nc.sync.dma_start(out=dbg_d["dbg_tT"][:, :], in_=tT[:, :])
                    nc.sync.dma_start(
                        out=dbg_d["dbg_ot"][:, :, :], in_=ot[:, :, :].bitcast(F32)
                    )
                    nc.sync.dma_start(out=dbg_d["dbg_qt"][:, :, :],
                                      in_=qt_all[:, :, 3 * QB:4 * QB])
                    nc.sync.dma_start(
                        out=dbg_d["dbg_kt"][:, :, :], in_=kt_sb[0][:, :, :]
                    )
                    nc.sync.dma_start(
                        out=dbg_d["dbg_v"][:, :, :], in_=v_sb[0][:, :, :]
                    )
                    nc.sync.dma_start(
                        out=dbg_d["dbg_pt"][:, :, :], in_=quarters[3][:, :, :]
                    )

    # Force Exp and Ln to resolve to the one table set containing both
    # (the default chooser alternates exp_and_others <-> natural_log_exp,
    # paying a ~1.3us table load per switch, ~17 loads per kernel).
    import bass_rust as _br
    from concourse.hw_specs import get_activation_tables as _gat

    def _patched_act_loads():
        has_act = any(
            isinstance(i, mybir.InstActivation)
            for blk in nc.main_func.blocks for i in blk.instructions
        )
        if not has_act:
            return
        tables = []
        for name, fns in _gat(nc.m.arch).items():
            if name != "natural_log_exp_and_others":
                fns = fns - {AF.Exp, AF.Ln}
            tables.append((name, fns))
        _br.insert_act_table_loads(nc, tables)

    nc.insert_act_table_loads = _patched_act_loads
    nc.compile()
    return nc


def get_nc(dbg=False):
    key = "nc_dbg" if dbg else "nc"
    if key not in _CACHE:
        _CACHE[key] = _build_nc(dbg)
    return _CACHE[key]


def make_in_maps(low, high, q_w, q_b, k_w, k_b, v_w, v_b, o_w, o_b, ln_g, ln_b):
    low_r = np.asarray(low, np.float32).reshape(B, C, N)
    high_r = np.asarray(high, np.float32).reshape(B, C, N)
    f32 = lambda x: np.ascontiguousarray(np.asarray(x, np.float32))
    # v-bias is exactly equivalent to an out-proj bias shift because the
    # softmax rows sum to one: attn @ (V + 1 vb^T) @ o_w^T = attn @ V @ o_w^T
    # + (o_w @ v_b)^T, so fold it on the host.
    ob_eff = np.asarray(o_b, np.float32) + np.asarray(o_w, np.float32) @ np.asarray(v_b, np.float32)
    pv_cols = []
    for v in [q_b, k_b, ob_eff, ln_g, ln_b]:
        pv_cols.append(np.asarray(v, np.float32).reshape(2, 128).T)
    shared = {
        "wq": f32(np.asarray(q_w, np.float32).T),
        "wk": f32(np.asarray(k_w, np.float32).T),
        "wv": f32(np.asarray(v_w, np.float32).T),
        "wo": f32(np.asarray(o_w, np.float32).T),
        "pvec": f32(np.concatenate(pv_cols, axis=1)),
    }
    in_maps = []
    for i in range(8):
        bidx, h = i // 2, i % 2
        in_maps.append({
            "low": f32(low_r[bidx][:, h * NQ:(h + 1) * NQ]),
            "high": f32(high_r[bidx]),
            **shared,
        })
    return in_maps


def assemble(results):
    out = np.empty((B, C, N), np.float32)
    for i in range(8):
        bidx, h = i // 2, i % 2
        out[bidx][:, h * NQ:(h + 1) * NQ] = results[i]["out"]
    return out.reshape(B, C, 64, 64)


def kernel(**inputs) -> np.ndarray:
    nc = get_nc()
    in_maps = make_in_maps(**inputs)
    res = run_bass_kernel_spmd(nc, in_maps, core_ids=list(range(8)))
    return assemble(res.results)


if __name__ == "__main__":
    pass



# revision 10
# speedup vs baseline: 1.0616x; 1.0616x over previous
"""ContentGuidedAttention Trainium2 kernel.

Full NxN single-head cross-attention + out-proj + residual + LayerNorm,
for B=4, C=256, H=W=64 (N=4096 tokens), distributed over 8 NeuronCores:
core i handles batch i//2, query-half i%2 (2048 queries, all 4096 keys).
No collectives: K/V are computed redundantly on the two cores sharing a
batch (~5% extra FLOPs).

Layout strategy (all channel-major, zero transposes):
  - Q^T, K^T computed as [C, n] (channels on partitions) in fp8e4
  - V computed token-major [n, C] in fp8e4
  - S^T = K Q^T via fp8 DoubleRow (K=256 contraction per MM, ~1.4x PE
    rate); exp on ACT -> P^T fp8e4
  - PV also fp8 DoubleRow (two 128-key chunks per MM)
  - softmax denominator: contiguous DVE chunk-tree then a ones-vector
    matmul reduces the 128 partitions -> [1, q]
  - reciprocals and rsqrt run on ACT as exp(-ln x) / exp(-0.5 ln x):
    Ln and Exp share one activation-table set, so no table switches
  - row -> all-partition replication via K=1 ones-column matmuls
  - PV: O^T[c, q] = sum_k V[k,c] P^T[k,q]; out-proj keeps channel-major
  - LN entirely per-query-block, overlapped with the next block's
    attention; affine via ACT Identity with per-partition scale/bias
Projection matmuls run in float32r (full PE rate at free dim >= 256).
"""

import numpy as np

import concourse.bass as bass
import concourse.mybir as mybir
import concourse.tile as tile
from concourse import bacc
from concourse.bass import ds, ts
from concourse.bass_utils import run_bass_kernel_spmd

F32 = mybir.dt.float32
F32R = mybir.dt.float32r
BF16 = mybir.dt.bfloat16
F8 = mybir.dt.float8e4
AF = mybir.ActivationFunctionType
OP = mybir.AluOpType
PM = mybir.MatmulPerfMode

B = 4
C = 256
N = 4096          # tokens per batch
NQ = 2048         # queries per core
QB = 512          # query block
NQB = NQ // QB    # 4
NKC = N // 128    # 32 key chunks
NKR = 4           # key ranges (1024 keys each) for K^T / V tiles
SCALE = (C // 8) ** -0.5
LN_EPS = 1e-5

_CACHE = {}


def _build_nc(dbg=False):
    nc = bacc.Bacc("TRN2", target_bir_lowering=False, debug=False)

    low_d = nc.declare_dram_parameter("low", [C, NQ], F32R, isOutput=False)
    high_d = nc.declare_dram_parameter("high", [C, N], F32R, isOutput=False)
    # weights are passed pre-transposed: [c_in, c_out]
    wq_d = nc.declare_dram_parameter("wq", [C, C], F32R, isOutput=False)
    wk_d = nc.declare_dram_parameter("wk", [C, C], F32R, isOutput=False)
    wv_d = nc.declare_dram_parameter("wv", [C, C], F32R, isOutput=False)
    wo_d = nc.declare_dram_parameter("wo", [C, C], F32R, isOutput=False)
    # qb, kb, ob, lng, lnb prepacked host-side as [128, 10]
    pvec_d = nc.declare_dram_parameter("pvec", [128, 10], F32, isOutput=False)
    out_d = nc.declare_dram_parameter("out", [C, NQ], F32, isOutput=True)
    dbg_d = {}
    if dbg:
        for nm, shp, dt_ in [
            ("dbg_rcp", [1, 512], F32), ("dbg_mu", [1, 512], F32),
            ("dbg_var", [1, 512], F32), ("dbg_rstd", [1, 512], F32),
            ("dbg_tT", [128, 512], BF16), ("dbg_ot", [128, 2, QB], F32),
            ("dbg_qt", [128, 2, QB], F8), ("dbg_kt", [128, 2, 1024], F8),
            ("dbg_v", [128, 8, C], F8), ("dbg_pt", [128, 8, QB], F8),
        ]:
            dbg_d[nm] = nc.declare_dram_parameter(nm, shp, dt_, isOutput=True)

    with tile.TileContext(nc) as tc:
        with (
            tc.tile_pool(name="persist", bufs=1) as pp,
            tc.tile_pool(name="high", bufs=3) as high_pool,
            tc.tile_pool(name="pt", bufs=9) as pt_pool,
            tc.tile_pool(name="ot", bufs=2) as ot_pool,
            tc.tile_pool(name="scratch", bufs=2) as scr_pool,
            tc.tile_pool(name="rowscr", bufs=1) as row_pool,
            tc.tile_pool(name="outsb", bufs=2) as out_pool,
            tc.tile_pool(name="st_ps", bufs=2, space="PSUM") as st_ps,
            tc.tile_pool(name="acc_ps", bufs=3, space="PSUM") as acc_ps,
            tc.tile_pool(name="row_ps", bufs=1, space="PSUM") as row_ps,
        ):
            # ---------------- constants / parameters ----------------
            # one tile per weight matrix ([cin_p, cin_chunk, cout]); K/V
            # weights load first so the K/V projections start ASAP
            pvec = pp.tile([128, 10], F32)
            nc.scalar.dma_start(out=pvec[:, :], in_=pvec_d[:, :])
            wk_sb = pp.tile([128, 2, C], F32R)
            wv_sb = pp.tile([128, 2, C], F32R)
            wq_sb = pp.tile([128, 2, C], F32R)
            wo_sb = pp.tile([128, 2, C], F32R)
            for t, d in [(wk_sb, wk_d), (wv_sb, wv_d), (wq_sb, wq_d),
                         (wo_sb, wo_d)]:
                for j in range(2):
                    nc.scalar.dma_start(out=t[:, j, :], in_=d[ds(j * 128, 128), :])

            # memset cannot emit float32r; stage in f32 and copy (the
            # DVE tensor_copy performs the f32 -> f32r rounding walrus wants)
            stage = pp.tile([128, 128], F32)
            ones1 = pp.tile([1, 128], F32R)      # K=1 replication lhsT
            nc.vector.memset(stage[ds(0, 1), :], 1.0)
            nc.vector.tensor_copy(ones1[:, :], stage[ds(0, 1), :])
            ones128 = pp.tile([128, 1], F32R)    # partition-reduce lhsT (f32r)
            nc.vector.memset(stage[:, 0:1], 1.0)
            nc.vector.tensor_copy(ones128[:, :], stage[:, 0:1])
            ones128b = pp.tile([128, 1], BF16)   # partition-reduce lhsT (bf16)
            nc.vector.memset(ones128b[:, :], 1.0)
            epsb = pp.tile([1, 1], F32)          # LN epsilon bias
            nc.vector.memset(epsb[:, :], LN_EPS)

            QBIAS, KBIAS, OBIAS, LNG, LNB = 0, 2, 4, 6, 8

            # ---------------- K^T / V projections ----------------
            # per 1024-key-range tiles so attention can start early
            kt_sb = [
                pp.tile([128, 2, 1024], F8, name=f"kt{r}", tag=f"kt{r}")
                for r in range(NKR)
            ]
            v_sb = [
                pp.tile([128, 8, C], F8, name=f"v{r}", tag=f"v{r}")
                for r in range(NKR)
            ]
            for kr in range(N // 512):
                hi = high_pool.tile([128, 2, 512], F32R)
                for j in range(2):
                    nc.sync.dma_start(
                        out=hi[:, j, :],
                        in_=high_d[ds(j * 128, 128), ds(kr * 512, 512)],
                    )
                r, h = kr // 2, kr % 2
                # K^T: out [cout, k] = sum_cin wk[cin, cout] high[cin, k]
                for c in range(2):
                    kps = st_ps.tile([128, 512], F32, tag="st")
                    for j in range(2):
                        nc.tensor.matmul(
                            out=kps[:, :],
                            lhsT=wk_sb[:, j, ds(c * 128, 128)],
                            rhs=hi[:, j, :],
                            start=(j == 0), stop=(j == 1),
                        )
                    nc.vector.tensor_scalar_add(
                        out=kt_sb[r][:, c, ds(h * 512, 512)],
                        in0=kps[:, :],
                        scalar1=pvec[:, ds(KBIAS + c, 1)],
                    )
                # V: out [k, cout] = sum_cin high[cin, k] wv[cin, cout]
                for u in range(4):
                    vps = st_ps.tile([128, C], F32, tag="st")
                    for j in range(2):
                        nc.tensor.matmul(
                            out=vps[:, :],
                            lhsT=hi[:, j, ds(u * 128, 128)],
                            rhs=wv_sb[:, j, :],
                            start=(j == 0), stop=(j == 1),
                        )
                    nc.scalar.activation(
                        out=v_sb[r][:, h * 4 + u, :], in_=vps[:, :],
                        func=AF.Copy,
                    )

            # ---------------- Q^T projection (all blocks) ----------------
            low_sb = pp.tile([128, 2, NQ], F32R)
            for j in range(2):
                nc.scalar.dma_start(
                    out=low_sb[:, j, :], in_=low_d[ds(j * 128, 128), :]
                )
            qt_all = pp.tile([128, 2, NQ], F8)
            for qb4 in range(NQB):
                for c in range(2):
                    qps = st_ps.tile([128, QB], F32, tag="st")
                    for j in range(2):
                        nc.tensor.matmul(
                            out=qps[:, :],
                            lhsT=wq_sb[:, j, ds(c * 128, 128)],
                            rhs=low_sb[:, j, ds(qb4 * QB, QB)],
                            start=(j == 0), stop=(j == 1),
                        )
                    nc.vector.tensor_scalar_add(
                        out=qt_all[:, c, ds(qb4 * QB, QB)], in0=qps[:, :],
                        scalar1=pvec[:, ds(QBIAS + c, 1)],
                    )


            # ---------------- main loop over query blocks ----------------
            # Emission is software-pipelined: block b's scalar-chain matmuls
            # (denominator, out-proj, LN stats) are emitted inside block
            # b+1's attention so the in-order PE queue never waits on the
            # DVE/ACT softmax-denominator and LayerNorm chains.

            def attention(b):
                qsl = ds(b * QB, QB)
                quarters = [
                    pt_pool.tile([128, 8, QB], F8, tag="ptq", name=f"ptq{g}")
                    for g in range(4)
                ]
                for si in range(16):
                    sps = st_ps.tile([128, 2, QB], F32, tag="st")
                    for u in range(2):
                        kc = si * 2 + u
                        # DoubleRow: full C=256 contraction in one fp8 MM
                        nc.tensor.matmul(
                            out=sps[:, u, :],
                            lhsT=kt_sb[kc // 8][:, :, ds((kc % 8) * 128, 128)],
                            rhs=qt_all[:, :, qsl],
                            start=True, stop=True,
                            perf_mode=PM.DoubleRow,
                        )
                    nc.scalar.activation(
                        out=quarters[si // 4][:, ds((si % 4) * 2, 2), :],
                        in_=sps[:, :, :],
                        func=AF.Exp,
                        scale=SCALE,
                    )
                return quarters

            def pv(b, quarters):
                ot = ot_pool.tile([128, 2, QB], F32R, tag="ot", name=f"ot{b}")
                for c in range(2):
                    ops = acc_ps.tile([128, QB], F32, tag="acc")
                    for t in range(NKC // 2):
                        # DoubleRow: two adjacent 128-key chunks per fp8 MM
                        nc.tensor.matmul(
                            out=ops[:, :],
                            lhsT=v_sb[t // 4][:, ds((t % 4) * 2, 2), ds(c * 128, 128)],
                            rhs=quarters[t // 4][:, ds((t % 4) * 2, 2), :],
                            start=(t == 0), stop=(t == NKC // 2 - 1),
                            perf_mode=PM.DoubleRow,
                        )
                    nc.vector.tensor_copy(ot[:, c, :], ops[:, :])
                return ot

            def tree(b, quarters):
                # contiguous DVE chunk tree for the softmax denominator.
                # quarters are fp8 now, so the first folds write bf16
                # scratch (fp8 lacks range for 32-chunk partial sums).
                # q0..q2 fold while exp of q3 is still streaming; q3 then
                # folds shallowly so the post-attention critical path is
                # only ~2us of DVE work.
                s8 = scr_pool.tile([128, 8, QB], BF16, tag="s8")
                nc.vector.tensor_add(
                    out=s8[:, :, :],
                    in0=quarters[0][:, :, :], in1=quarters[1][:, :, :],
                )
                nc.vector.tensor_add(
                    out=s8[:, :, :],
                    in0=s8[:, :, :], in1=quarters[2][:, :, :],
                )
                tB = scr_pool.tile([128, 4, QB], BF16, tag="tB")
                nc.vector.tensor_add(
                    out=tB[:, :, :],
                    in0=s8[:, 0:4, :], in1=s8[:, 4:8, :],
                )
                tT2 = scr_pool.tile([128, 2, QB], BF16, tag="tT2")
                nc.vector.tensor_add(
                    out=tT2[:, :, :], in0=tB[:, 0:2, :], in1=tB[:, 2:4, :]
                )
                p012 = scr_pool.tile([128, QB], BF16, tag="p012")
                nc.vector.tensor_add(
                    out=p012[:, :], in0=tT2[:, 0, :], in1=tT2[:, 1, :]
                )
                a3 = scr_pool.tile([128, 4, QB], BF16, tag="tB")
                nc.vector.tensor_add(
                    out=a3[:, :, :],
                    in0=quarters[3][:, 0:4, :], in1=quarters[3][:, 4:8, :],
                )
                b3 = scr_pool.tile([128, 2, QB], BF16, tag="tT2")
                nc.vector.tensor_add(
                    out=b3[:, :, :], in0=a3[:, 0:2, :], in1=a3[:, 2:4, :]
                )
                tT = scr_pool.tile([128, QB], BF16, tag="tT", name=f"tT{b}")
                nc.vector.scalar_tensor_tensor(
                    out=tT[:, :], in0=b3[:, 0, :], scalar=0.0,
                    in1=b3[:, 1, :], op0=OP.add, op1=OP.add,
                )
                nc.vector.tensor_add(
                    out=tT[:, :], in0=tT[:, :], in1=p012[:, :]
                )
                return tT

            def denom_recip(b, tT):
                dn_ps = row_ps.tile([1, QB], F32, tag="row")
                nc.tensor.matmul(
                    out=dn_ps[:, :], lhsT=ones128b[:, :], rhs=tT[:, :],
                    start=True, stop=True,
                )
                # 1/denom = exp(-ln(denom)) on ACT (same table set as Exp)
                lnrow = row_pool.tile([1, QB], F32, tag="lnrow")
                nc.scalar.activation(
                    out=lnrow[:, :], in_=dn_ps[:, :], func=AF.Ln
                )
                rcprow = row_pool.tile([1, QB], F32, tag="rcprow",
                                       name=f"rcprow{b}")
                nc.scalar.activation(
                    out=rcprow[:, :], in_=lnrow[:, :], func=AF.Exp, scale=-1.0
                )
                rcp_rep = scr_pool.tile([128, QB], F32, tag="rcprep",
                                        name=f"rcprep{b}")
                nc.gpsimd.partition_broadcast(rcp_rep[:, :], rcprow[:, :])
                return rcprow, rcp_rep

            def outproj_y(b, ot, rcp_rep):
                qsl = ds(b * QB, QB)
                y_sb = ot_pool.tile([128, 2, QB], F32R, tag="y", name=f"y{b}")
                for c in range(2):
                    pps = acc_ps.tile([128, QB], F32, tag="acc")
                    for j in range(2):
                        nc.tensor.matmul(
                            out=pps[:, :],
                            lhsT=wo_sb[:, j, ds(c * 128, 128)],
                            rhs=ot[:, j, :],
                            start=(j == 0), stop=(j == 1),
                        )
                    ysc = scr_pool.tile([128, QB], F32, tag="scr")
                    nc.vector.tensor_mul(
                        out=ysc[:, :], in0=pps[:, :], in1=rcp_rep[:, :]
                    )
                    nc.vector.scalar_tensor_tensor(
                        out=y_sb[:, c, :],
                        in0=ysc[:, :],
                        scalar=pvec[:, ds(OBIAS + c, 1)],
                        in1=low_sb[:, c, qsl].bitcast(F32),
                        op0=OP.add, op1=OP.add,
                    )
                return y_sb

            def stats_ln(b, y_sb, rcprow):
                qsl = ds(b * QB, QB)
                sy_ps = row_ps.tile([1, QB], F32, tag="row")
                for c in range(2):
                    nc.tensor.matmul(
                        out=sy_ps[:, :],
                        lhsT=ones128[:, :],
                        rhs=y_sb[:, c, :],
                        start=(c == 0), stop=(c == 1),
                    )
                murow = row_pool.tile([1, QB], F32, tag="murow")
                nc.vector.tensor_scalar_mul(
                    out=murow[:, :], in0=sy_ps[:, :], scalar1=1.0 / C
                )
                sy2_ps = row_ps.tile([1, QB], F32, tag="row")
                for c in range(2):
                    ysq = scr_pool.tile([128, QB], F32R, tag="ysq")
                    nc.vector.tensor_mul(
                        out=ysq[:, :],
                        in0=y_sb[:, c, :].bitcast(F32),
                        in1=y_sb[:, c, :].bitcast(F32),
                    )
                    nc.tensor.matmul(
                        out=sy2_ps[:, :],
                        lhsT=ones128[:, :],
                        rhs=ysq[:, :],
                        start=(c == 0), stop=(c == 1),
                    )
                # var = E[y^2] - mu^2 ; rstd = exp(-0.5 ln(var + eps))
                varrow = row_pool.tile([1, QB], F32, tag="varrow")
                nc.vector.tensor_scalar_mul(
                    out=varrow[:, :], in0=sy2_ps[:, :], scalar1=1.0 / C
                )
                mu2row = row_pool.tile([1, QB], F32, tag="mu2row")
                nc.vector.tensor_mul(
                    out=mu2row[:, :], in0=murow[:, :], in1=murow[:, :],
                )
                nc.vector.tensor_sub(
                    out=varrow[:, :], in0=varrow[:, :], in1=mu2row[:, :]
                )
                lnv = row_pool.tile([1, QB], F32, tag="lnv")
                nc.scalar.activation(
                    out=lnv[:, :], in_=varrow[:, :], func=AF.Ln, bias=epsb[:, :]
                )
                rstdrow = row_pool.tile([1, QB], F32, tag="rstdrow")
                nc.scalar.activation(
                    out=rstdrow[:, :], in_=lnv[:, :], func=AF.Exp, scale=-0.5
                )
                if dbg_d and b == NQB - 1:
                    nc.sync.dma_start(out=dbg_d["dbg_rcp"][:, :], in_=rcprow[:, :])
                    nc.sync.dma_start(out=dbg_d["dbg_mu"][:, :], in_=murow[:, :])
                    nc.sync.dma_start(out=dbg_d["dbg_var"][:, :],
                                      in_=varrow[:, :])
                    nc.sync.dma_start(out=dbg_d["dbg_rstd"][:, :],
                                      in_=rstdrow[:, :])
                mu_rep = scr_pool.tile([128, QB], F32, tag="murep")
                nc.gpsimd.partition_broadcast(mu_rep[:, :], murow[:, :])
                rs_rep = scr_pool.tile([128, QB], F32, tag="rsrep")
                nc.gpsimd.partition_broadcast(rs_rep[:, :], rstdrow[:, :])
                for c in range(2):
                    yn = scr_pool.tile([128, QB], F32, tag="scr")
                    nc.vector.tensor_sub(
                        out=yn[:, :],
                        in0=y_sb[:, c, :].bitcast(F32),
                        in1=mu_rep[:, :],
                    )
                    nc.vector.tensor_mul(
                        out=yn[:, :], in0=yn[:, :], in1=rs_rep[:, :]
                    )
                    osb = out_pool.tile([128, QB], F32)
                    nc.vector.tensor_scalar(
                        out=osb[:, :], in0=yn[:, :],
                        scalar1=pvec[:, ds(LNG + c, 1)],
                        scalar2=pvec[:, ds(LNB + c, 1)],
                        op0=OP.mult, op1=OP.add,
                    )
                    nc.scalar.dma_start(
                        out=out_d[ds(c * 128, 128), qsl], in_=osb[:, :]
                    )

            for b in range(NQB):
                quarters = attention(b)
                ot = pv(b, quarters)
                tT = tree(b, quarters)
                rcprow, rcp_rep = denom_recip(b, tT)
                y_b = outproj_y(b, ot, rcp_rep)
                stats_ln(b, y_b, rcprow)
                if dbg_d and b == NQB - 1:
                    nc.sync.dma_start(out=dbg_d["dbg_tT"][:, :], in_=tT[:, :])
                    nc.sync.dma_start(
                        out=dbg_d["dbg_ot"][:, :, :], in_=ot[:, :, :].bitcast(F32)
                    )
                    nc.sync.dma_start(out=dbg_d["dbg_qt"][:, :, :],
                                      in_=qt_all[:, :, 3 * QB:4 * QB])
                    nc.sync.dma_start(
                        out=dbg_d["dbg_kt"][:, :, :], in_=kt_sb[0][:, :, :]
                    )
                    nc.sync.dma_start(
                        out=dbg_d["dbg_v"][:, :, :], in_=v_sb[0][:, :, :]
                    )
                    nc.sync.dma_start(
                        out=dbg_d["dbg_pt"][:, :, :], in_=quarters[3][:, :, :]
                    )

    # Force Exp and Ln to resolve to the one table set containing both
    # (the default chooser alternates exp_and_others <-> natural_log_exp,
    # paying a ~1.3us table load per switch, ~17 loads per kernel).
    import bass_rust as _br
    from concourse.hw_specs import get_activation_tables as _gat

    def _patched_act_loads():
        has_act = any(
            isinstance(i, mybir.InstActivation)
            for blk in nc.main_func.blocks for i in blk.instructions
        )
        if not has_act:
            return
        tables = []
        for name, fns in _gat(nc.m.arch).items():
            if name != "natural_log_exp_and_others":
                fns = fns - {AF.Exp, AF.Ln}
            tables.append((name, fns))
        _br.insert_act_table_loads(nc, tables)

    nc.insert_act_table_loads = _patched_act_loads
    nc.compile()
    return nc


def get_nc(dbg=False):
    key = "nc_dbg" if dbg else "nc"
    if key not in _CACHE:
        _CACHE[key] = _build_nc(dbg)
    return _CACHE[key]


def make_in_maps(low, high, q_w, q_b, k_w, k_b, v_w, v_b, o_w, o_b, ln_g, ln_b):
    low_r = np.asarray(low, np.float32).reshape(B, C, N)
    high_r = np.asarray(high, np.float32).reshape(B, C, N)
    f32 = lambda x: np.ascontiguousarray(np.asarray(x, np.float32))
    # v-bias is exactly equivalent to an out-proj bias shift because the
    # softmax rows sum to one: attn @ (V + 1 vb^T) @ o_w^T = attn @ V @ o_w^T
    # + (o_w @ v_b)^T, so fold it on the host.
    ob_eff = np.asarray(o_b, np.float32) + np.asarray(o_w, np.float32) @ np.asarray(v_b, np.float32)
    pv_cols = []
    for v in [q_b, k_b, ob_eff, ln_g, ln_b]:
        pv_cols.append(np.asarray(v, np.float32).reshape(2, 128).T)
    shared = {
        "wq": f32(np.asarray(q_w, np.float32).T),
        "wk": f32(np.asarray(k_w, np.float32).T),
        "wv": f32(np.asarray(v_w, np.float32).T),
        "wo": f32(np.asarray(o_w, np.float32).T),
        "pvec": f32(np.concatenate(pv_cols, axis=1)),
    }
    in_maps = []
    for i in range(8):
        bidx, h = i // 2, i % 2
        in_maps.append({
            "low": f32(low_r[bidx][:, h * NQ:(h + 1) * NQ]),
            "high": f32(high_r[bidx]),
            **shared,
        })
    return in_maps


def assemble(results):
    out = np.empty((B, C, N), np.float32)
    for i in range(8):
        bidx, h = i // 2, i % 2
        out[bidx][:, h * NQ:(h + 1) * NQ] = results[i]["out"]
    return out.reshape(B, C, 64, 64)


def kernel(**inputs) -> np.ndarray:
    nc = get_nc()
    in_maps = make_in_maps(**inputs)
    res = run_bass_kernel_spmd(nc, in_maps, core_ids=list(range(8)))
    return assemble(res.results)


if __name__ == "__main__":
    pass



# revision 11
# speedup vs baseline: 1.2506x; 1.1779x over previous
"""ContentGuidedAttention Trainium2 kernel.

Full NxN single-head cross-attention + out-proj + residual + LayerNorm,
for B=4, C=256, H=W=64 (N=4096 tokens), distributed over 8 NeuronCores:
core i handles batch i//2, query-half i%2 (2048 queries, all 4096 keys).
No collectives: K/V are computed redundantly on the two cores sharing a
batch (~5% extra FLOPs).

Layout strategy (all channel-major, zero transposes):
  - the out-projection is folded into V host-side: W_vo = (o_w @ v_w),
    so PV directly yields the projected output; V is scaled by 64 (and
    Q/K weights by 16) to keep fp8e4 out of subnormals, with the scales
    folded back via the exp scale and the denominator ones-vector
  - Q^T, K^T computed as [C, n] (channels on partitions) in fp8e4
  - V' computed token-major [n, C] in fp8e4
  - all projections and attention matmuls run fp8 DoubleRow (K=256
    contraction per MM, ~1.45x bf16 PE rate)
  - S^T = K Q^T as [k, q] psum tiles; exp on ACT -> P^T fp8e4
  - softmax denominator: quarters 0-1 via DVE bf16 chunk-tree,
    quarters 2-3 via fp8 DoubleRow ones-matmuls, all accumulated in
    one [1, q] psum group (the ones carry the 64x V scale)
  - reciprocals and rsqrt run on ACT as exp(-ln x) / exp(-0.5 ln x):
    Ln and Exp share one activation-table set, so no table switches
  - row -> all-partition replication via GpSimd partition broadcast
  - LN per-query-block, overlapped with the next block's attention
"""

import numpy as np

import concourse.bass as bass
import concourse.mybir as mybir
import concourse.tile as tile
from concourse import bacc
from concourse.bass import ds, ts
from concourse.bass_utils import run_bass_kernel_spmd

F32 = mybir.dt.float32
F32R = mybir.dt.float32r
BF16 = mybir.dt.bfloat16
F8 = mybir.dt.float8e4
AF = mybir.ActivationFunctionType
OP = mybir.AluOpType
PM = mybir.MatmulPerfMode

B = 4
C = 256
N = 4096          # tokens per batch
NQ = 2048         # queries per core
QB = 512          # query block
NQB = NQ // QB    # 4
NKC = N // 128    # 32 key chunks
NKR = 4           # key ranges (1024 keys each) for K^T / V tiles
SQK = 16.0        # host-side scale on wq/wk (fp8 subnormal avoidance)
SV = 64.0         # host-side scale on wvo
SCALE = (C // 8) ** -0.5 / (SQK * SQK)
LN_EPS = 1e-5

_CACHE = {}


def _build_nc(dbg=False):
    nc = bacc.Bacc("TRN2", target_bir_lowering=False, debug=False)

    low_d = nc.declare_dram_parameter("low", [C, NQ], F32R, isOutput=False)
    lowq_d = nc.declare_dram_parameter("lowq", [C, NQ], F8, isOutput=False)
    high_d = nc.declare_dram_parameter("high", [C, N], F8, isOutput=False)
    # weights pre-transposed [c_in, c_out]; wvo = (o_w @ v_w).T * 64
    wq_d = nc.declare_dram_parameter("wq", [C, C], F8, isOutput=False)
    wk_d = nc.declare_dram_parameter("wk", [C, C], F8, isOutput=False)
    wvo_d = nc.declare_dram_parameter("wvo", [C, C], F8, isOutput=False)
    # qb16, kb16, ob_eff, lng, lnb prepacked host-side as [128, 10]
    pvec_d = nc.declare_dram_parameter("pvec", [128, 10], F32, isOutput=False)
    out_d = nc.declare_dram_parameter("out", [C, NQ], F32, isOutput=True)
    dbg_d = {}
    if dbg:
        for nm, shp, dt_ in [
            ("dbg_rcp", [1, 512], F32), ("dbg_mu", [1, 512], F32),
            ("dbg_var", [1, 512], F32), ("dbg_rstd", [1, 512], F32),
            ("dbg_pps", [128, 2, QB], F32),
            ("dbg_qt", [128, 2, QB], F8), ("dbg_kt", [128, 2, 1024], F8),
            ("dbg_v", [128, 8, C], F8), ("dbg_pt", [128, 8, QB], F8),
        ]:
            dbg_d[nm] = nc.declare_dram_parameter(nm, shp, dt_, isOutput=True)

    with tile.TileContext(nc) as tc:
        with (
            tc.tile_pool(name="persist", bufs=1) as pp,
            tc.tile_pool(name="high", bufs=3) as high_pool,
            tc.tile_pool(name="pt", bufs=9) as pt_pool,
            tc.tile_pool(name="yt", bufs=2) as yt_pool,
            tc.tile_pool(name="scratch", bufs=2) as scr_pool,
            tc.tile_pool(name="rowscr", bufs=1) as row_pool,
            tc.tile_pool(name="outsb", bufs=2) as out_pool,
            tc.tile_pool(name="st_ps", bufs=2, space="PSUM") as st_ps,
            tc.tile_pool(name="acc_ps", bufs=2, space="PSUM") as acc_ps,
            tc.tile_pool(name="row_ps", bufs=2, space="PSUM") as row_ps,
        ):
            # ---------------- constants / parameters ----------------
            # K/V weights load first so the K/V projections start ASAP
            pvec = pp.tile([128, 10], F32)
            nc.gpsimd.dma_start(out=pvec[:, :], in_=pvec_d[:, :])
            wk_sb = pp.tile([128, 2, C], F8)
            wvo_sb = pp.tile([128, 2, C], F8)
            wq_sb = pp.tile([128, 2, C], F8)
            for t, d in [(wk_sb, wk_d), (wvo_sb, wvo_d), (wq_sb, wq_d)]:
                for j in range(2):
                    nc.scalar.dma_start(out=t[:, j, :], in_=d[ds(j * 128, 128), :])

            # memset cannot emit float32r/fp8; stage in f32 and copy
            stage = pp.tile([128, 128], F32)
            ones128 = pp.tile([128, 1], F32R)    # partition-reduce lhsT (f32r)
            nc.vector.memset(stage[:, 0:1], 1.0)
            nc.vector.tensor_copy(ones128[:, :], stage[:, 0:1])
            # denominator lhsT vectors carry the 64x V scale
            ones128b = pp.tile([128, 1], BF16)   # bf16 tree-row reduce
            nc.vector.memset(ones128b[:, :], SV)
            ones_f8 = pp.tile([128, 2, 16], F8)  # fp8 DoubleRow ones (col 0)
            nc.vector.memset(stage[:, 0:32], SV)
            nc.vector.tensor_copy(
                ones_f8[:, :, :], stage[:, 0:32].rearrange("p (a b) -> p a b", a=2)
            )
            epsb = pp.tile([1, 1], F32)          # LN epsilon bias
            nc.vector.memset(epsb[:, :], LN_EPS)

            QBIAS, KBIAS, OBIAS, LNG, LNB = 0, 2, 4, 6, 8

            # ---------------- K^T / V' projections ----------------
            # per 1024-key-range tiles so attention can start early
            kt_sb = [
                pp.tile([128, 2, 1024], F8, name=f"kt{r}", tag=f"kt{r}")
                for r in range(NKR)
            ]
            v_sb = [
                pp.tile([128, 8, C], F8, name=f"v{r}", tag=f"v{r}")
                for r in range(NKR)
            ]
            for kr in range(N // 512):
                hi = high_pool.tile([128, 2, 512], F8)
                for j in range(2):
                    nc.sync.dma_start(
                        out=hi[:, j, :],
                        in_=high_d[ds(j * 128, 128), ds(kr * 512, 512)],
                    )
                r, h = kr // 2, kr % 2
                # K^T: out [cout, k] = sum_cin wk[cin, cout] high[cin, k]
                for c in range(2):
                    kps = st_ps.tile([128, 512], F32, tag="st")
                    nc.tensor.matmul(
                        out=kps[:, :],
                        lhsT=wk_sb[:, :, ds(c * 128, 128)],
                        rhs=hi[:, :, :],
                        start=True, stop=True,
                        perf_mode=PM.DoubleRow,
                    )
                    nc.vector.tensor_scalar_add(
                        out=kt_sb[r][:, c, ds(h * 512, 512)],
                        in0=kps[:, :],
                        scalar1=pvec[:, ds(KBIAS + c, 1)],
                    )
                # V': out [k, cout] = sum_cin high[cin, k] wvo[cin, cout]
                for u in range(4):
                    vps = st_ps.tile([128, C], F32, tag="st")
                    nc.tensor.matmul(
                        out=vps[:, :],
                        lhsT=hi[:, :, ds(u * 128, 128)],
                        rhs=wvo_sb[:, :, :],
                        start=True, stop=True,
                        perf_mode=PM.DoubleRow,
                    )
                    nc.scalar.activation(
                        out=v_sb[r][:, h * 4 + u, :], in_=vps[:, :],
                        func=AF.Copy,
                    )

            # ---------------- Q^T projection (all blocks) ----------------
            low_sb = pp.tile([128, 2, NQ], F32R)
            lowq_sb = pp.tile([128, 2, NQ], F8)
            for j in range(2):
                nc.gpsimd.dma_start(
                    out=lowq_sb[:, j, :], in_=lowq_d[ds(j * 128, 128), :]
                )
            for j in range(2):
                nc.gpsimd.dma_start(
                    out=low_sb[:, j, :], in_=low_d[ds(j * 128, 128), :]
                )
            qt_all = pp.tile([128, 2, NQ], F8)
            for qb4 in range(NQB):
                for c in range(2):
                    qps = st_ps.tile([128, QB], F32, tag="st")
                    nc.tensor.matmul(
                        out=qps[:, :],
                        lhsT=wq_sb[:, :, ds(c * 128, 128)],
                        rhs=lowq_sb[:, :, ds(qb4 * QB, QB)],
                        start=True, stop=True,
                        perf_mode=PM.DoubleRow,
                    )
                    nc.vector.tensor_scalar_add(
                        out=qt_all[:, c, ds(qb4 * QB, QB)], in0=qps[:, :],
                        scalar1=pvec[:, ds(QBIAS + c, 1)],
                    )

            # ---------------- main loop over query blocks ----------------

            def attention(b):
                qsl = ds(b * QB, QB)
                quarters = [
                    pt_pool.tile([128, 8, QB], F8, tag="ptq", name=f"ptq{g}")
                    for g in range(4)
                ]
                for si in range(16):
                    sps = st_ps.tile([128, 2, QB], F32, tag="st")
                    for u in range(2):
                        kc = si * 2 + u
                        # DoubleRow: full C=256 contraction in one fp8 MM
                        nc.tensor.matmul(
                            out=sps[:, u, :],
                            lhsT=kt_sb[kc // 8][:, :, ds((kc % 8) * 128, 128)],
                            rhs=qt_all[:, :, qsl],
                            start=True, stop=True,
                            perf_mode=PM.DoubleRow,
                        )
                    nc.scalar.activation(
                        out=quarters[si // 4][:, ds((si % 4) * 2, 2), :],
                        in_=sps[:, :, :],
                        func=AF.Exp,
                        scale=SCALE,
                    )
                return quarters

            def pv(b, quarters):
                pps = []
                for c in range(2):
                    ops = acc_ps.tile([128, QB], F32, tag="acc")
                    for t in range(NKC // 2):
                        # DoubleRow: two adjacent 128-key chunks per fp8 MM
                        nc.tensor.matmul(
                            out=ops[:, :],
                            lhsT=v_sb[t // 4][:, ds((t % 4) * 2, 2), ds(c * 128, 128)],
                            rhs=quarters[t // 4][:, ds((t % 4) * 2, 2), :],
                            start=(t == 0), stop=(t == NKC // 2 - 1),
                            perf_mode=PM.DoubleRow,
                        )
                    pps.append(ops)
                return pps

            def denom(b, quarters):
                # quarters 0-1: DVE bf16 chunk tree (runs while exp of
                # q2/q3 streams); quarters 2-3: fp8 ones-matmuls. All
                # land in one [1, QB] psum accumulation group; the 64x
                # lhsT values fold the V' scale into the denominator.
                s8 = scr_pool.tile([128, 8, QB], BF16, tag="s8")
                nc.vector.tensor_add(
                    out=s8[:, :, :],
                    in0=quarters[0][:, :, :], in1=quarters[1][:, :, :],
                )
                tB = scr_pool.tile([128, 4, QB], BF16, tag="tB")
                nc.vector.tensor_add(
                    out=tB[:, :, :], in0=s8[:, 0:4, :], in1=s8[:, 4:8, :]
                )
                tT2 = scr_pool.tile([128, 2, QB], BF16, tag="tT2")
                nc.vector.tensor_add(
                    out=tT2[:, :, :], in0=tB[:, 0:2, :], in1=tB[:, 2:4, :]
                )
                tT = scr_pool.tile([128, QB], BF16, tag="tT", name=f"tT{b}")
                nc.vector.tensor_add(
                    out=tT[:, :], in0=tT2[:, 0, :], in1=tT2[:, 1, :]
                )
                dn_ps = row_ps.tile([1, QB], F32, tag="row")
                nmm = 9
                i = 0
                for g in (2, 3):
                    for t in range(4):
                        nc.tensor.matmul(
                            out=dn_ps[:, :],
                            lhsT=ones_f8[:, :, 0:1],
                            rhs=quarters[g][:, ds(t * 2, 2), :],
                            start=(i == 0), stop=False,
                            perf_mode=PM.DoubleRow,
                        )
                        i += 1
                nc.tensor.matmul(
                    out=dn_ps[:, :], lhsT=ones128b[:, :], rhs=tT[:, :],
                    start=False, stop=True,
                )
                # 1/denom = exp(-ln(denom)) on ACT (same table set as Exp)
                lnrow = row_pool.tile([1, QB], F32, tag="lnrow")
                nc.scalar.activation(
                    out=lnrow[:, :], in_=dn_ps[:, :], func=AF.Ln
                )
                rcprow = row_pool.tile([1, QB], F32, tag="rcprow",
                                       name=f"rcprow{b}")
                nc.scalar.activation(
                    out=rcprow[:, :], in_=lnrow[:, :], func=AF.Exp, scale=-1.0
                )
                rcp_rep = scr_pool.tile([128, QB], F32, tag="rcprep",
                                        name=f"rcprep{b}")
                nc.gpsimd.partition_broadcast(rcp_rep[:, :], rcprow[:, :])
                return rcprow, rcp_rep

            def make_y(b, pps, rcp_rep):
                qsl = ds(b * QB, QB)
                y_sb = yt_pool.tile([128, 2, QB], F32R, tag="y", name=f"y{b}")
                for c in range(2):
                    ysc = scr_pool.tile([128, QB], F32, tag="scr")
                    nc.vector.tensor_mul(
                        out=ysc[:, :], in0=pps[c][:, :], in1=rcp_rep[:, :]
                    )
                    nc.vector.scalar_tensor_tensor(
                        out=y_sb[:, c, :],
                        in0=ysc[:, :],
                        scalar=pvec[:, ds(OBIAS + c, 1)],
                        in1=low_sb[:, c, qsl].bitcast(F32),
                        op0=OP.add, op1=OP.add,
                    )
                return y_sb

            def stats_ln(b, y_sb, rcprow):
                sy_ps = row_ps.tile([1, QB], F32, tag="row")
                for c in range(2):
                    nc.tensor.matmul(
                        out=sy_ps[:, :],
                        lhsT=ones128[:, :],
                        rhs=y_sb[:, c, :],
                        start=(c == 0), stop=(c == 1),
                    )
                murow = row_pool.tile([1, QB], F32, tag="murow")
                nc.vector.tensor_scalar_mul(
                    out=murow[:, :], in0=sy_ps[:, :], scalar1=1.0 / C
                )
                sy2_ps = row_ps.tile([1, QB], F32, tag="row")
                for c in range(2):
                    ysq = scr_pool.tile([128, QB], F32R, tag="ysq")
                    nc.vector.tensor_mul(
                        out=ysq[:, :],
                        in0=y_sb[:, c, :].bitcast(F32),
                        in1=y_sb[:, c, :].bitcast(F32),
                    )
                    nc.tensor.matmul(
                        out=sy2_ps[:, :],
                        lhsT=ones128[:, :],
                        rhs=ysq[:, :],
                        start=(c == 0), stop=(c == 1),
                    )
                # var = E[y^2] - mu^2 ; rstd = exp(-0.5 ln(var + eps))
                varrow = row_pool.tile([1, QB], F32, tag="varrow")
                nc.vector.tensor_scalar_mul(
                    out=varrow[:, :], in0=sy2_ps[:, :], scalar1=1.0 / C
                )
                mu2row = row_pool.tile([1, QB], F32, tag="mu2row")
                nc.vector.tensor_mul(
                    out=mu2row[:, :], in0=murow[:, :], in1=murow[:, :],
                )
                nc.vector.tensor_sub(
                    out=varrow[:, :], in0=varrow[:, :], in1=mu2row[:, :]
                )
                lnv = row_pool.tile([1, QB], F32, tag="lnv")
                nc.scalar.activation(
                    out=lnv[:, :], in_=varrow[:, :], func=AF.Ln, bias=epsb[:, :]
                )
                rstdrow = row_pool.tile([1, QB], F32, tag="rstdrow")
                nc.scalar.activation(
                    out=rstdrow[:, :], in_=lnv[:, :], func=AF.Exp, scale=-0.5
                )
                if dbg_d and b == NQB - 1:
                    nc.sync.dma_start(out=dbg_d["dbg_rcp"][:, :], in_=rcprow[:, :])
                    nc.sync.dma_start(out=dbg_d["dbg_mu"][:, :], in_=murow[:, :])
                    nc.sync.dma_start(out=dbg_d["dbg_var"][:, :],
                                      in_=varrow[:, :])
                    nc.sync.dma_start(out=dbg_d["dbg_rstd"][:, :],
                                      in_=rstdrow[:, :])
                mu_rep = scr_pool.tile([128, QB], F32, tag="murep")
                nc.gpsimd.partition_broadcast(mu_rep[:, :], murow[:, :])
                rs_rep = scr_pool.tile([128, QB], F32, tag="rsrep")
                nc.gpsimd.partition_broadcast(rs_rep[:, :], rstdrow[:, :])
                qsl = ds(b * QB, QB)
                for c in range(2):
                    yn = scr_pool.tile([128, QB], F32, tag="scr")
                    nc.vector.tensor_sub(
                        out=yn[:, :],
                        in0=y_sb[:, c, :].bitcast(F32),
                        in1=mu_rep[:, :],
                    )
                    nc.vector.tensor_mul(
                        out=yn[:, :], in0=yn[:, :], in1=rs_rep[:, :]
                    )
                    osb = out_pool.tile([128, QB], F32)
                    nc.vector.tensor_scalar(
                        out=osb[:, :], in0=yn[:, :],
                        scalar1=pvec[:, ds(LNG + c, 1)],
                        scalar2=pvec[:, ds(LNB + c, 1)],
                        op0=OP.mult, op1=OP.add,
                    )
                    nc.gpsimd.dma_start(
                        out=out_d[ds(c * 128, 128), qsl], in_=osb[:, :]
                    )

            for b in range(NQB):
                quarters = attention(b)
                pps = pv(b, quarters)
                rcprow, rcp_rep = denom(b, quarters)
                y_b = make_y(b, pps, rcp_rep)
                stats_ln(b, y_b, rcprow)
                if dbg_d and b == NQB - 1:
                    nc.sync.dma_start(
                        out=dbg_d["dbg_pps"][:, 0, :], in_=pps[0][:, :]
                    )
                    nc.sync.dma_start(out=dbg_d["dbg_qt"][:, :, :],
                                      in_=qt_all[:, :, 3 * QB:4 * QB])
                    nc.sync.dma_start(
                        out=dbg_d["dbg_kt"][:, :, :], in_=kt_sb[0][:, :, :]
                    )
                    nc.sync.dma_start(
                        out=dbg_d["dbg_v"][:, :, :], in_=v_sb[0][:, :, :]
                    )
                    nc.sync.dma_start(
                        out=dbg_d["dbg_pt"][:, :, :], in_=quarters[3][:, :, :]
                    )

    # Force Exp and Ln to resolve to the one table set containing both
    # (the default chooser alternates exp_and_others <-> natural_log_exp,
    # paying a ~1.3us table load per switch, ~17 loads per kernel).
    import bass_rust as _br
    from concourse.hw_specs import get_activation_tables as _gat

    def _patched_act_loads():
        has_act = any(
            isinstance(i, mybir.InstActivation)
            for blk in nc.main_func.blocks for i in blk.instructions
        )
        if not has_act:
            return
        tables = []
        for name, fns in _gat(nc.m.arch).items():
            if name != "natural_log_exp_and_others":
                fns = fns - {AF.Exp, AF.Ln}
            tables.append((name, fns))
        _br.insert_act_table_loads(nc, tables)

    nc.insert_act_table_loads = _patched_act_loads
    nc.compile()
    return nc


def get_nc(dbg=False):
    key = "nc_dbg" if dbg else "nc"
    if key not in _CACHE:
        _CACHE[key] = _build_nc(dbg)
    return _CACHE[key]


def make_in_maps(low, high, q_w, q_b, k_w, k_b, v_w, v_b, o_w, o_b, ln_g, ln_b):
    import ml_dtypes
    f32 = lambda x: np.ascontiguousarray(np.asarray(x, np.float32))
    f8 = lambda x: np.ascontiguousarray(
        np.asarray(x, np.float32).astype(ml_dtypes.float8_e4m3)
    )
    low_r = np.asarray(low, np.float32).reshape(B, C, N)
    high_r = np.asarray(high, np.float32).reshape(B, C, N)
    # v-bias is exactly equivalent to an out-proj bias shift because the
    # softmax rows sum to one: attn @ (V + 1 vb^T) @ o_w^T = attn @ V @ o_w^T
    # + (o_w @ v_b)^T, so fold it on the host. The out-projection itself
    # folds into V: attn @ V @ o_w^T = attn @ (high_t @ (o_w @ v_w).T).
    o_w = np.asarray(o_w, np.float32)
    v_w = np.asarray(v_w, np.float32)
    ob_eff = np.asarray(o_b, np.float32) + o_w @ np.asarray(v_b, np.float32)
    w_vo = (o_w @ v_w) * SV
    pv_cols = []
    for v in [np.asarray(q_b, np.float32) * SQK,
              np.asarray(k_b, np.float32) * SQK, ob_eff, ln_g, ln_b]:
        pv_cols.append(np.asarray(v, np.float32).reshape(2, 128).T)
    shared = {
        "wq": f8(np.asarray(q_w, np.float32).T * SQK),
        "wk": f8(np.asarray(k_w, np.float32).T * SQK),
        "wvo": f8(w_vo.T),
        "pvec": f32(np.concatenate(pv_cols, axis=1)),
    }
    in_maps = []
    for i in range(8):
        bidx, h = i // 2, i % 2
        in_maps.append({
            "low": f32(low_r[bidx][:, h * NQ:(h + 1) * NQ]),
            "lowq": f8(low_r[bidx][:, h * NQ:(h + 1) * NQ]),
            "high": f8(high_r[bidx]),
            **shared,
        })
    return in_maps


def assemble(results):
    out = np.empty((B, C, N), np.float32)
    for i in range(8):
        bidx, h = i // 2, i % 2
        out[bidx][:, h * NQ:(h + 1) * NQ] = results[i]["out"]
    return out.reshape(B, C, 64, 64)


def kernel(**inputs) -> np.ndarray:
    nc = get_nc()
    in_maps = make_in_maps(**inputs)
    res = run_bass_kernel_spmd(nc, in_maps, core_ids=list(range(8)))
    return assemble(res.results)


if __name__ == "__main__":
    pass


# revision 15
# speedup vs baseline: 1.2895x; 1.0311x over previous
"""ContentGuidedAttention Trainium2 kernel.

Full NxN single-head cross-attention + out-proj + residual + LayerNorm,
for B=4, C=256, H=W=64 (N=4096 tokens), distributed over 8 NeuronCores:
core i handles batch i//2, query-half i%2 (2048 queries, all 4096 keys).
No collectives: K/V are computed redundantly on the two cores sharing a
batch (~5% extra FLOPs).

Layout strategy (all channel-major, zero transposes):
  - the out-projection is folded into V host-side: W_vo = (o_w @ v_w),
    so PV directly yields the projected output; V is scaled by 64 (and
    Q/K weights by 16) to keep fp8e4 out of subnormals, with the scales
    folded back via the exp scale and the denominator ones-vector
  - Q^T, K^T computed as [C, n] (channels on partitions) in fp8e4
  - V' computed token-major [n, C] in fp8e4
  - all projections and attention matmuls run fp8 DoubleRow (K=256
    contraction per MM, ~1.45x bf16 PE rate)
  - S^T = K Q^T as [k, q] psum tiles; exp on ACT -> P^T fp8e4
  - softmax denominator: quarters 0-1 via DVE bf16 chunk-tree,
    quarters 2-3 via fp8 DoubleRow ones-matmuls, all accumulated in
    one [1, q] psum group (the ones carry the 64x V scale)
  - reciprocals and rsqrt run on ACT as exp(-ln x) / exp(-0.5 ln x):
    Ln and Exp share one activation-table set, so no table switches
  - row -> all-partition replication via GpSimd partition broadcast
  - LN per-query-block, overlapped with the next block's attention
"""

import numpy as np

import concourse.bass as bass
import concourse.mybir as mybir
import concourse.tile as tile
from concourse import bacc
from concourse.bass import ds, ts
from concourse.bass_utils import run_bass_kernel_spmd

F32 = mybir.dt.float32
F32R = mybir.dt.float32r
BF16 = mybir.dt.bfloat16
F8 = mybir.dt.float8e4
AF = mybir.ActivationFunctionType
OP = mybir.AluOpType
PM = mybir.MatmulPerfMode

B = 4
C = 256
N = 4096          # tokens per batch
NQ = 2048         # queries per core
QB = 512          # query block
NQB = NQ // QB    # 4
NKC = N // 128    # 32 key chunks
NKR = 4           # key ranges (1024 keys each) for K^T / V tiles
SQK = 16.0        # host-side scale on wq/wk (fp8 subnormal avoidance)
SV = 64.0         # host-side scale on wvo
SCALE = (C // 8) ** -0.5 / (SQK * SQK)
LN_EPS = 1e-5

_CACHE = {}


def _build_nc(dbg=False):
    nc = bacc.Bacc("TRN2", target_bir_lowering=False, debug=False)

    low_d = nc.declare_dram_parameter("low", [C, NQ], F32R, isOutput=False)
    lowq_d = nc.declare_dram_parameter("lowq", [C, NQ], F8, isOutput=False)
    high_d = nc.declare_dram_parameter("high", [C, N], F8, isOutput=False)
    # weights pre-transposed [c_in, c_out]; wvo = (o_w @ v_w).T * 64
    wq_d = nc.declare_dram_parameter("wq", [C, C], F8, isOutput=False)
    wk_d = nc.declare_dram_parameter("wk", [C, C], F8, isOutput=False)
    wvo_d = nc.declare_dram_parameter("wvo", [C, C], F8, isOutput=False)
    # qb16, kb16, ob_eff, lng, lnb prepacked host-side as [128, 10]
    pvec_d = nc.declare_dram_parameter("pvec", [128, 10], F32, isOutput=False)
    out_d = nc.declare_dram_parameter("out", [C, NQ], F32, isOutput=True)
    dbg_d = {}
    if dbg:
        for nm, shp, dt_ in [
            ("dbg_rcp", [1, 512], F32), ("dbg_mu", [1, 512], F32),
            ("dbg_var", [1, 512], F32), ("dbg_rstd", [1, 512], F32),
            ("dbg_pps", [128, 2, QB], F32),
            ("dbg_qt", [128, 2, QB], F8), ("dbg_kt", [128, 2, 1024], F8),
            ("dbg_v", [128, 8, C], F8), ("dbg_pt", [128, 8, QB], F8),
        ]:
            dbg_d[nm] = nc.declare_dram_parameter(nm, shp, dt_, isOutput=True)

    with tile.TileContext(nc) as tc:
        with (
            tc.tile_pool(name="persist", bufs=1) as pp,
            tc.tile_pool(name="high", bufs=3) as high_pool,
            tc.tile_pool(name="pt", bufs=9) as pt_pool,
            tc.tile_pool(name="yt", bufs=2) as yt_pool,
            tc.tile_pool(name="scratch", bufs=2) as scr_pool,
            tc.tile_pool(name="rowscr", bufs=1) as row_pool,
            tc.tile_pool(name="outsb", bufs=2) as out_pool,
            tc.tile_pool(name="st_ps", bufs=2, space="PSUM") as st_ps,
            tc.tile_pool(name="acc_ps", bufs=3, space="PSUM") as acc_ps,
            tc.tile_pool(name="row_ps", bufs=1, space="PSUM") as row_ps,
        ):
            # ---------------- constants / parameters ----------------
            # single-descriptor DMAs: the [256, n] DRAM halves fold into
            # [128, 2, n] SBUF tiles via AP rearrange, one post each
            pvec = pp.tile([128, 10], F32)
            nc.sync.dma_start(out=pvec[:, :], in_=pvec_d[:, :])
            wk_sb = pp.tile([128, 2, C], F8)
            wq_sb = pp.tile([128, 2, C], F8)
            wvo_sb = pp.tile([128, 2, C], F8)
            for t, d in [(wk_sb, wk_d), (wq_sb, wq_d), (wvo_sb, wvo_d)]:
                nc.scalar.dma_start(
                    out=t[:, :, :],
                    in_=d[:, :].rearrange("(j p) k -> p j k", j=2),
                )
            lowq_sb = pp.tile([128, 2, NQ], F8)
            nc.sync.dma_start(
                out=lowq_sb[:, :, :],
                in_=lowq_d[:, :].rearrange("(j p) k -> p j k", j=2),
            )
            low_sb = pp.tile([128, 2, NQ], F32R)
            nc.sync.dma_start(
                out=low_sb[:, :, :],
                in_=low_d[:, :].rearrange("(j p) k -> p j k", j=2),
            )

            # memset cannot emit float32r/fp8; stage in f32 and copy
            stage = pp.tile([128, 128], F32)
            ones128 = pp.tile([128, 1], F32R)    # partition-reduce lhsT (f32r)
            nc.vector.memset(stage[:, 0:1], 1.0)
            nc.vector.tensor_copy(ones128[:, :], stage[:, 0:1])
            # denominator lhsT carries the 64x V scale
            ones_f8 = pp.tile([128, 2, 16], F8)  # fp8 DoubleRow ones (col 0)
            nc.vector.memset(stage[:, 0:32], SV)
            nc.vector.tensor_copy(
                ones_f8[:, :, :], stage[:, 0:32].rearrange("p (a b) -> p a b", a=2)
            )
            epsb = pp.tile([1, 1], F32)          # LN epsilon bias
            nc.vector.memset(epsb[:, :], LN_EPS)

            QBIAS, KBIAS, OBIAS, LNG, LNB = 0, 2, 4, 6, 8

            # ---------------- projections ----------------
            # hi tiles persist so K^T / Q^T / V' emission can be staged:
            # Q^T right after K^T so block 0's attention starts early;
            # V' (needed only ~15us later, by PV(0)) emitted after.
            hi_sb = [
                pp.tile([128, 2, 1024], F8, name=f"hi{r}", tag=f"hi{r}")
                for r in range(NKR)
            ]
            for r in range(NKR):
                nc.sync.dma_start(
                    out=hi_sb[r][:, :, :],
                    in_=high_d[:, ds(r * 1024, 1024)].rearrange(
                        "(j p) k -> p j k", j=2
                    ),
                )
            kt_sb = [
                pp.tile([128, 2, 1024], F8, name=f"kt{r}", tag=f"kt{r}")
                for r in range(NKR)
            ]
            v_sb = [
                pp.tile([128, 8, C], F8, name=f"v{r}", tag=f"v{r}")
                for r in range(NKR)
            ]
            # K^T: out [cout, k] = sum_cin wk[cin, cout] high[cin, k]
            for r in range(NKR):
                for h in range(2):
                    for c in range(2):
                        kps = st_ps.tile([128, 512], F32, tag="st")
                        nc.tensor.matmul(
                            out=kps[:, :],
                            lhsT=wk_sb[:, :, ds(c * 128, 128)],
                            rhs=hi_sb[r][:, :, ds(h * 512, 512)],
                            start=True, stop=True,
                            perf_mode=PM.DoubleRow,
                        )
                        nc.vector.tensor_scalar_add(
                            out=kt_sb[r][:, c, ds(h * 512, 512)],
                            in0=kps[:, :],
                            scalar1=pvec[:, ds(KBIAS + c, 1)],
                        )
            # Q^T projection (all blocks)
            qt_all = pp.tile([128, 2, NQ], F8)
            for qb4 in range(NQB):
                for c in range(2):
                    qps = st_ps.tile([128, QB], F32, tag="st")
                    nc.tensor.matmul(
                        out=qps[:, :],
                        lhsT=wq_sb[:, :, ds(c * 128, 128)],
                        rhs=lowq_sb[:, :, ds(qb4 * QB, QB)],
                        start=True, stop=True,
                        perf_mode=PM.DoubleRow,
                    )
                    nc.vector.tensor_scalar_add(
                        out=qt_all[:, c, ds(qb4 * QB, QB)], in0=qps[:, :],
                        scalar1=pvec[:, ds(QBIAS + c, 1)],
                    )
            # V': out [k, cout] = sum_cin high[cin, k] wvo[cin, cout]
            for r in range(NKR):
                for u in range(8):
                    vps = st_ps.tile([128, C], F32, tag="st")
                    nc.tensor.matmul(
                        out=vps[:, :],
                        lhsT=hi_sb[r][:, :, ds(u * 128, 128)],
                        rhs=wvo_sb[:, :, :],
                        start=True, stop=True,
                        perf_mode=PM.DoubleRow,
                    )
                    nc.scalar.activation(
                        out=v_sb[r][:, u, :], in_=vps[:, :],
                        func=AF.Copy,
                    )

            # ---------------- main loop over query blocks ----------------

            def attention(b):
                qsl = ds(b * QB, QB)
                quarters = [
                    pt_pool.tile([128, 8, QB], F8, tag="ptq", name=f"ptq{g}")
                    for g in range(4)
                ]
                for si in range(16):
                    sps = st_ps.tile([128, 2, QB], F32, tag="st")
                    for u in range(2):
                        kc = si * 2 + u
                        # DoubleRow: full C=256 contraction in one fp8 MM
                        nc.tensor.matmul(
                            out=sps[:, u, :],
                            lhsT=kt_sb[kc // 8][:, :, ds((kc % 8) * 128, 128)],
                            rhs=qt_all[:, :, qsl],
                            start=True, stop=True,
                            perf_mode=PM.DoubleRow,
                        )
                    nc.scalar.activation(
                        out=quarters[si // 4][:, ds((si % 4) * 2, 2), :],
                        in_=sps[:, :, :],
                        func=AF.Exp,
                        scale=SCALE,
                    )
                return quarters

            def pv(b, quarters):
                pps = []
                for c in range(2):
                    ops = acc_ps.tile([128, QB], F32, tag="acc")
                    for t in range(NKC // 2):
                        # DoubleRow: two adjacent 128-key chunks per fp8 MM
                        nc.tensor.matmul(
                            out=ops[:, :],
                            lhsT=v_sb[t // 4][:, ds((t % 4) * 2, 2), ds(c * 128, 128)],
                            rhs=quarters[t // 4][:, ds((t % 4) * 2, 2), :],
                            start=(t == 0), stop=(t == NKC // 2 - 1),
                            perf_mode=PM.DoubleRow,
                        )
                    pps.append(ops)
                return pps

            def denom(b, quarters):
                # softmax denominator: fp8 DoubleRow ones-matmuls over
                # every quarter pair, one [1, QB] psum accumulation
                # group; the 64x lhsT values fold in the V' scale.
                # Purely exp-gated (no DVE dependency), so the next
                # block's S matmuls aren't stalled behind DVE folds.
                dn_ps = row_ps.tile([1, QB], F32, tag="row")
                for i in range(16):
                    nc.tensor.matmul(
                        out=dn_ps[:, :],
                        lhsT=ones_f8[:, :, 0:1],
                        rhs=quarters[i // 4][:, ds((i % 4) * 2, 2), :],
                        start=(i == 0), stop=(i == 15),
                        perf_mode=PM.DoubleRow,
                    )
                # 1/denom = exp(-ln(denom)) on ACT (same table set as Exp)
                lnrow = row_pool.tile([1, QB], F32, tag="lnrow")
                nc.scalar.activation(
                    out=lnrow[:, :], in_=dn_ps[:, :], func=AF.Ln
                )
                rcprow = row_pool.tile([1, QB], F32, tag="rcprow",
                                       name=f"rcprow{b}")
                nc.scalar.activation(
                    out=rcprow[:, :], in_=lnrow[:, :], func=AF.Exp, scale=-1.0
                )
                rcp_rep = scr_pool.tile([128, QB], F32, tag="rcprep",
                                        name=f"rcprep{b}")
                nc.gpsimd.partition_broadcast(rcp_rep[:, :], rcprow[:, :])
                return rcprow, rcp_rep

            def make_y(b, pps, rcp_rep):
                qsl = ds(b * QB, QB)
                y_sb = yt_pool.tile([128, 2, QB], F32R, tag="y", name=f"y{b}")
                for c in range(2):
                    ysc = scr_pool.tile([128, QB], F32, tag="scr")
                    nc.vector.tensor_mul(
                        out=ysc[:, :], in0=pps[c][:, :], in1=rcp_rep[:, :]
                    )
                    nc.vector.scalar_tensor_tensor(
                        out=y_sb[:, c, :],
                        in0=ysc[:, :],
                        scalar=pvec[:, ds(OBIAS + c, 1)],
                        in1=low_sb[:, c, qsl].bitcast(F32),
                        op0=OP.add, op1=OP.add,
                    )
                return y_sb

            def stats_ln(b, y_sb, rcprow):
                sy_ps = row_ps.tile([1, QB], F32, tag="row")
                for c in range(2):
                    nc.tensor.matmul(
                        out=sy_ps[:, :],
                        lhsT=ones128[:, :],
                        rhs=y_sb[:, c, :],
                        start=(c == 0), stop=(c == 1),
                    )
                murow = row_pool.tile([1, QB], F32, tag="murow")
                nc.vector.tensor_scalar_mul(
                    out=murow[:, :], in0=sy_ps[:, :], scalar1=1.0 / C
                )
                sy2_ps = row_ps.tile([1, QB], F32, tag="row")
                for c in range(2):
                    ysq = scr_pool.tile([128, QB], F32R, tag="ysq")
                    nc.vector.tensor_mul(
                        out=ysq[:, :],
                        in0=y_sb[:, c, :].bitcast(F32),
                        in1=y_sb[:, c, :].bitcast(F32),
                    )
                    nc.tensor.matmul(
                        out=sy2_ps[:, :],
                        lhsT=ones128[:, :],
                        rhs=ysq[:, :],
                        start=(c == 0), stop=(c == 1),
                    )
                # var = E[y^2] - mu^2 ; rstd = exp(-0.5 ln(var + eps))
                varrow = row_pool.tile([1, QB], F32, tag="varrow")
                nc.vector.tensor_scalar_mul(
                    out=varrow[:, :], in0=sy2_ps[:, :], scalar1=1.0 / C
                )
                mu2row = row_pool.tile([1, QB], F32, tag="mu2row")
                nc.vector.tensor_mul(
                    out=mu2row[:, :], in0=murow[:, :], in1=murow[:, :],
                )
                nc.vector.tensor_sub(
                    out=varrow[:, :], in0=varrow[:, :], in1=mu2row[:, :]
                )
                lnv = row_pool.tile([1, QB], F32, tag="lnv")
                nc.scalar.activation(
                    out=lnv[:, :], in_=varrow[:, :], func=AF.Ln, bias=epsb[:, :]
                )
                rstdrow = row_pool.tile([1, QB], F32, tag="rstdrow")
                nc.scalar.activation(
                    out=rstdrow[:, :], in_=lnv[:, :], func=AF.Exp, scale=-0.5
                )
                if dbg_d and b == NQB - 1:
                    nc.sync.dma_start(out=dbg_d["dbg_rcp"][:, :], in_=rcprow[:, :])
                    nc.sync.dma_start(out=dbg_d["dbg_mu"][:, :], in_=murow[:, :])
                    nc.sync.dma_start(out=dbg_d["dbg_var"][:, :],
                                      in_=varrow[:, :])
                    nc.sync.dma_start(out=dbg_d["dbg_rstd"][:, :],
                                      in_=rstdrow[:, :])
                mu_rep = scr_pool.tile([128, QB], F32, tag="murep")
                nc.gpsimd.partition_broadcast(mu_rep[:, :], murow[:, :])
                rs_rep = scr_pool.tile([128, QB], F32, tag="rsrep")
                nc.gpsimd.partition_broadcast(rs_rep[:, :], rstdrow[:, :])
                qsl = ds(b * QB, QB)
                osb = out_pool.tile([128, 2, QB], F32)
                for c in range(2):
                    yn = scr_pool.tile([128, QB], F32, tag="scr")
                    nc.vector.tensor_sub(
                        out=yn[:, :],
                        in0=y_sb[:, c, :].bitcast(F32),
                        in1=mu_rep[:, :],
                    )
                    nc.vector.tensor_mul(
                        out=yn[:, :], in0=yn[:, :], in1=rs_rep[:, :]
                    )
                    nc.vector.tensor_scalar(
                        out=osb[:, c, :], in0=yn[:, :],
                        scalar1=pvec[:, ds(LNG + c, 1)],
                        scalar2=pvec[:, ds(LNB + c, 1)],
                        op0=OP.mult, op1=OP.add,
                    )
                nc.sync.dma_start(
                    out=out_d[:, qsl].rearrange("(j p) q -> p j q", j=2),
                    in_=osb[:, :, :],
                )

            for b in range(NQB):
                quarters = attention(b)
                pps = pv(b, quarters)
                rcprow, rcp_rep = denom(b, quarters)
                y_b = make_y(b, pps, rcp_rep)
                stats_ln(b, y_b, rcprow)
                if dbg_d and b == NQB - 1:
                    nc.sync.dma_start(
                        out=dbg_d["dbg_pps"][:, 0, :], in_=pps[0][:, :]
                    )
                    nc.sync.dma_start(out=dbg_d["dbg_qt"][:, :, :],
                                      in_=qt_all[:, :, 3 * QB:4 * QB])
                    nc.sync.dma_start(
                        out=dbg_d["dbg_kt"][:, :, :], in_=kt_sb[0][:, :, :]
                    )
                    nc.sync.dma_start(
                        out=dbg_d["dbg_v"][:, :, :], in_=v_sb[0][:, :, :]
                    )
                    nc.sync.dma_start(
                        out=dbg_d["dbg_pt"][:, :, :], in_=quarters[3][:, :, :]
                    )

    # Force Exp and Ln to resolve to the one table set containing both
    # (the default chooser alternates exp_and_others <-> natural_log_exp,
    # paying a ~1.3us table load per switch, ~17 loads per kernel).
    import bass_rust as _br
    from concourse.hw_specs import get_activation_tables as _gat

    def _patched_act_loads():
        has_act = any(
            isinstance(i, mybir.InstActivation)
            for blk in nc.main_func.blocks for i in blk.instructions
        )
        if not has_act:
            return
        tables = []
        for name, fns in _gat(nc.m.arch).items():
            if name != "natural_log_exp_and_others":
                fns = fns - {AF.Exp, AF.Ln}
            tables.append((name, fns))
        _br.insert_act_table_loads(nc, tables)

    nc.insert_act_table_loads = _patched_act_loads
    nc.compile()
    return nc


def get_nc(dbg=False):
    key = "nc_dbg" if dbg else "nc"
    if key not in _CACHE:
        _CACHE[key] = _build_nc(dbg)
    return _CACHE[key]


def make_in_maps(low, high, q_w, q_b, k_w, k_b, v_w, v_b, o_w, o_b, ln_g, ln_b):
    import ml_dtypes
    f32 = lambda x: np.ascontiguousarray(np.asarray(x, np.float32))
    f8 = lambda x: np.ascontiguousarray(
        np.asarray(x, np.float32).astype(ml_dtypes.float8_e4m3)
    )
    low_r = np.asarray(low, np.float32).reshape(B, C, N)
    high_r = np.asarray(high, np.float32).reshape(B, C, N)
    # v-bias is exactly equivalent to an out-proj bias shift because the
    # softmax rows sum to one: attn @ (V + 1 vb^T) @ o_w^T = attn @ V @ o_w^T
    # + (o_w @ v_b)^T, so fold it on the host. The out-projection itself
    # folds into V: attn @ V @ o_w^T = attn @ (high_t @ (o_w @ v_w).T).
    o_w = np.asarray(o_w, np.float32)
    v_w = np.asarray(v_w, np.float32)
    ob_eff = np.asarray(o_b, np.float32) + o_w @ np.asarray(v_b, np.float32)
    w_vo = (o_w @ v_w) * SV
    pv_cols = []
    for v in [np.asarray(q_b, np.float32) * SQK,
              np.asarray(k_b, np.float32) * SQK, ob_eff, ln_g, ln_b]:
        pv_cols.append(np.asarray(v, np.float32).reshape(2, 128).T)
    shared = {
        "wq": f8(np.asarray(q_w, np.float32).T * SQK),
        "wk": f8(np.asarray(k_w, np.float32).T * SQK),
        "wvo": f8(w_vo.T),
        "pvec": f32(np.concatenate(pv_cols, axis=1)),
    }
    in_maps = []
    for i in range(8):
        bidx, h = i // 2, i % 2
        in_maps.append({
            "low": f32(low_r[bidx][:, h * NQ:(h + 1) * NQ]),
            "lowq": f8(low_r[bidx][:, h * NQ:(h + 1) * NQ]),
            "high": f8(high_r[bidx]),
            **shared,
        })
    return in_maps


def assemble(results):
    out = np.empty((B, C, N), np.float32)
    for i in range(8):
        bidx, h = i // 2, i % 2
        out[bidx][:, h * NQ:(h + 1) * NQ] = results[i]["out"]
    return out.reshape(B, C, 64, 64)


def kernel(**inputs) -> np.ndarray:
    nc = get_nc()
    in_maps = make_in_maps(**inputs)
    res = run_bass_kernel_spmd(nc, in_maps, core_ids=list(range(8)))
    return assemble(res.results)


if __name__ == "__main__":
    pass


# revision 18
# speedup vs baseline: 1.4742x; 1.1433x over previous
"""ContentGuidedAttention Trainium2 kernel.

Full NxN single-head cross-attention + out-proj + residual + LayerNorm,
for B=4, C=256, H=W=64 (N=4096 tokens), distributed over 8 NeuronCores:
core i handles batch i//2, query-half i%2 (2048 queries, all 4096 keys).
No collectives: K/V are computed redundantly on the two cores sharing a
batch (~5% extra FLOPs).

Layout strategy (all channel-major, zero transposes):
  - the out-projection is folded into V host-side: W_vo = (o_w @ v_w),
    so PV directly yields the projected output; V is scaled by 64 (and
    Q/K weights by 16) to keep fp8e4 out of subnormals, with the scales
    folded back via the exp scale and the denominator ones-vector
  - Q^T, K^T computed as [C, n] (channels on partitions) in fp8e4
  - V' computed token-major [n, C] in fp8e4
  - all projections and attention matmuls run fp8 DoubleRow (K=256
    contraction per MM, ~1.45x bf16 PE rate)
  - S^T = K Q^T as [k, q] psum tiles; exp on ACT -> P^T fp8e4
  - softmax denominator: quarters 0-1 via DVE bf16 chunk-tree,
    quarters 2-3 via fp8 DoubleRow ones-matmuls, all accumulated in
    one [1, q] psum group (the ones carry the 64x V scale)
  - reciprocals and rsqrt run on ACT as exp(-ln x) / exp(-0.5 ln x):
    Ln and Exp share one activation-table set, so no table switches
  - row -> all-partition replication via GpSimd partition broadcast
  - LN per-query-block, overlapped with the next block's attention
"""

import numpy as np

import concourse.bass as bass
import concourse.mybir as mybir
import concourse.tile as tile
from concourse import bacc
from concourse.bass import ds, ts
from concourse.bass_utils import run_bass_kernel_spmd

F32 = mybir.dt.float32
F32R = mybir.dt.float32r
BF16 = mybir.dt.bfloat16
F8 = mybir.dt.float8e4
AF = mybir.ActivationFunctionType
OP = mybir.AluOpType
PM = mybir.MatmulPerfMode

B = 4
C = 256
N = 4096          # tokens per batch
NQ = 2048         # queries per core
QB = 512          # query block
NQB = NQ // QB    # 4
NKC = N // 128    # 32 key chunks
NKR = 4           # key ranges (1024 keys each) for K^T / V tiles
SQK = 16.0        # host-side scale on wq/wk (fp8 subnormal avoidance)
SV = 64.0         # host-side scale on wvo
SCALE = (C // 8) ** -0.5 / (SQK * SQK)
LN_EPS = 1e-5

_CACHE = {}


def _build_nc(dbg=False):
    nc = bacc.Bacc("TRN2", target_bir_lowering=False, debug=False)

    low_d = nc.declare_dram_parameter("low", [C, NQ], F32R, isOutput=False)
    lowq_d = nc.declare_dram_parameter("lowq", [C, NQ], F8, isOutput=False)
    high_d = nc.declare_dram_parameter("high", [C, N], F8, isOutput=False)
    # weights pre-transposed [c_in, c_out]; wvo = (o_w @ v_w).T * 64
    wq_d = nc.declare_dram_parameter("wq", [C, C], F8, isOutput=False)
    wk_d = nc.declare_dram_parameter("wk", [C, C], F8, isOutput=False)
    wvo_d = nc.declare_dram_parameter("wvo", [C, C], F8, isOutput=False)
    # qb16, kb16, ob_eff, lng, lnb prepacked host-side as [128, 10]
    pvec_d = nc.declare_dram_parameter("pvec", [128, 10], F32, isOutput=False)
    out_d = nc.declare_dram_parameter("out", [C, NQ], F32, isOutput=True)
    dbg_d = {}
    if dbg:
        for nm, shp, dt_ in [
            ("dbg_rcp", [1, 512], F32), ("dbg_mu", [1, 512], F32),
            ("dbg_var", [1, 512], F32), ("dbg_rstd", [1, 512], F32),
            ("dbg_pps", [128, 2, QB], F32),
            ("dbg_qt", [128, 2, QB], F8), ("dbg_kt", [128, 2, 1024], F8),
            ("dbg_v", [128, 8, C], F8), ("dbg_pt", [128, 8, QB], F8),
        ]:
            dbg_d[nm] = nc.declare_dram_parameter(nm, shp, dt_, isOutput=True)

    with tile.TileContext(nc) as tc:
        with (
            tc.tile_pool(name="persist", bufs=1) as pp,
            tc.tile_pool(name="high", bufs=3) as high_pool,
            tc.tile_pool(name="pt", bufs=9) as pt_pool,
            tc.tile_pool(name="yt", bufs=2) as yt_pool,
            tc.tile_pool(name="scratch", bufs=2) as scr_pool,
            tc.tile_pool(name="rowscr", bufs=1) as row_pool,
            tc.tile_pool(name="outsb", bufs=2) as out_pool,
            tc.tile_pool(name="st_ps", bufs=2, space="PSUM") as st_ps,
            tc.tile_pool(name="acc_ps", bufs=3, space="PSUM") as acc_ps,
            tc.tile_pool(name="row_ps", bufs=1, space="PSUM") as row_ps,
        ):
            # ---------------- constants / parameters ----------------
            # single-descriptor DMAs: the [256, n] DRAM halves fold into
            # [128, 2, n] SBUF tiles via AP rearrange, one post each
            pvec = pp.tile([128, 10], F32)
            nc.sync.dma_start(out=pvec[:, :], in_=pvec_d[:, :])
            # DMA order matters: hi chunks first (gate the K proj and
            # everything after), then lowq; the big f32 low residual
            # rides the scalar queue after the weights (needed ~25us in)
            hi_sb = [
                pp.tile([128, 2, 1024], F8, name=f"hi{r}", tag=f"hi{r}")
                for r in range(NKR)
            ]
            wk_sb = pp.tile([128, 2, C], F8)
            wq_sb = pp.tile([128, 2, C], F8)
            wvo_sb = pp.tile([128, 2, C], F8)
            for t, d in [(wk_sb, wk_d), (wq_sb, wq_d), (wvo_sb, wvo_d)]:
                nc.scalar.dma_start(
                    out=t[:, :, :],
                    in_=d[:, :].rearrange("(j p) k -> p j k", j=2),
                )
            for r in range(NKR):
                nc.sync.dma_start(
                    out=hi_sb[r][:, :, :],
                    in_=high_d[:, ds(r * 1024, 1024)].rearrange(
                        "(j p) k -> p j k", j=2
                    ),
                )
            lowq_sb = pp.tile([128, 2, NQ], F8)
            nc.sync.dma_start(
                out=lowq_sb[:, :, :],
                in_=lowq_d[:, :].rearrange("(j p) k -> p j k", j=2),
            )
            low_sb = pp.tile([128, 2, NQ], F32R)
            nc.scalar.dma_start(
                out=low_sb[:, :, :],
                in_=low_d[:, :].rearrange("(j p) k -> p j k", j=2),
            )

            # memset cannot emit float32r/fp8; stage in f32 and copy
            stage = pp.tile([128, 128], F32)
            ones128 = pp.tile([128, 1], F32R)    # partition-reduce lhsT (f32r)
            nc.vector.memset(stage[:, 0:1], 1.0)
            nc.vector.tensor_copy(ones128[:, :], stage[:, 0:1])
            # denominator lhsT carries the 64x V scale
            ones_f8 = pp.tile([128, 2, 16], F8)  # fp8 DoubleRow ones (col 0)
            nc.vector.memset(stage[:, 0:32], SV)
            nc.vector.tensor_copy(
                ones_f8[:, :, :], stage[:, 0:32].rearrange("p (a b) -> p a b", a=2)
            )
            epsb = pp.tile([1, 1], F32)          # LN epsilon bias
            nc.vector.memset(epsb[:, :], LN_EPS)

            # HAM warm-up: dummy f32 matmuls fill the DMA-wait window so
            # the PE clock-gate opens before the first real matmul
            warm_ps = row_ps.tile([1, 128], F32, tag="row")
            for w in range(10):
                nc.tensor.matmul(
                    out=warm_ps[:, :], lhsT=stage[:, 0:1], rhs=stage[:, :],
                    start=True, stop=True,
                )

            QBIAS, KBIAS, OBIAS, LNG, LNB = 0, 2, 4, 6, 8

            # ---------------- projections ----------------
            # psum tiles alternate between the st/acc pools so the
            # projections pace at PE speed, not DVE bias-add speed.
            kt_sb = [
                pp.tile([128, 2, 1024], F8, name=f"kt{r}", tag=f"kt{r}")
                for r in range(NKR)
            ]
            v_sb = [
                pp.tile([128, 8, C], F8, name=f"v{r}", tag=f"v{r}")
                for r in range(NKR)
            ]
            qt_all = pp.tile([128, 2, NQ], F8)

            def proj_psum(i, shape):
                pool = (st_ps, acc_ps, acc_ps)[i % 3]
                return pool.tile(
                    shape, F32, tag="st" if i % 3 == 0 else "acc",
                    name=f"pps{i}",
                )

            def k_proj():
                # K^T: out [cout, k] = sum_cin wk[cin, cout] high[cin, k]
                i = 0
                for r in range(NKR):
                    for h in range(2):
                        for c in range(2):
                            kps = proj_psum(i, [128, 512])
                            i += 1
                            nc.tensor.matmul(
                                out=kps[:, :],
                                lhsT=wk_sb[:, :, ds(c * 128, 128)],
                                rhs=hi_sb[r][:, :, ds(h * 512, 512)],
                                start=True, stop=True,
                                perf_mode=PM.DoubleRow,
                            )
                            nc.vector.tensor_scalar_add(
                                out=kt_sb[r][:, c, ds(h * 512, 512)],
                                in0=kps[:, :],
                                scalar1=pvec[:, ds(KBIAS + c, 1)],
                            )

            def q_proj():
                i = 0
                for qb4 in range(NQB):
                    for c in range(2):
                        qps = proj_psum(i, [128, QB])
                        i += 1
                        nc.tensor.matmul(
                            out=qps[:, :],
                            lhsT=wq_sb[:, :, ds(c * 128, 128)],
                            rhs=lowq_sb[:, :, ds(qb4 * QB, QB)],
                            start=True, stop=True,
                            perf_mode=PM.DoubleRow,
                        )
                        nc.vector.tensor_scalar_add(
                            out=qt_all[:, c, ds(qb4 * QB, QB)], in0=qps[:, :],
                            scalar1=pvec[:, ds(QBIAS + c, 1)],
                        )

            def v_proj():
                # V': out [k, cout] = sum_cin high[cin, k] wvo[cin, cout]
                # copies on DVE: ACT is already saturated by block 0 exp
                i = 0
                for r in range(NKR):
                    for u in range(8):
                        vps = proj_psum(i, [128, C])
                        i += 1
                        nc.tensor.matmul(
                            out=vps[:, :],
                            lhsT=hi_sb[r][:, :, ds(u * 128, 128)],
                            rhs=wvo_sb[:, :, :],
                            start=True, stop=True,
                            perf_mode=PM.DoubleRow,
                        )
                        nc.vector.tensor_copy(v_sb[r][:, u, :], vps[:, :])

            # ---------------- main loop over query blocks ----------------

            def attention(b):
                qsl = ds(b * QB, QB)
                quarters = [
                    pt_pool.tile([128, 8, QB], F8, tag="ptq", name=f"ptq{g}")
                    for g in range(4)
                ]
                for si in range(16):
                    sps = st_ps.tile([128, 2, QB], F32, tag="st")
                    for u in range(2):
                        kc = si * 2 + u
                        # DoubleRow: full C=256 contraction in one fp8 MM
                        nc.tensor.matmul(
                            out=sps[:, u, :],
                            lhsT=kt_sb[kc // 8][:, :, ds((kc % 8) * 128, 128)],
                            rhs=qt_all[:, :, qsl],
                            start=True, stop=True,
                            perf_mode=PM.DoubleRow,
                        )
                    nc.scalar.activation(
                        out=quarters[si // 4][:, ds((si % 4) * 2, 2), :],
                        in_=sps[:, :, :],
                        func=AF.Exp,
                        scale=SCALE,
                    )
                return quarters

            def pv(b, quarters):
                pps = []
                for c in range(2):
                    ops = acc_ps.tile([128, QB], F32, tag="acc")
                    for t in range(NKC // 2):
                        # DoubleRow: two adjacent 128-key chunks per fp8 MM
                        nc.tensor.matmul(
                            out=ops[:, :],
                            lhsT=v_sb[t // 4][:, ds((t % 4) * 2, 2), ds(c * 128, 128)],
                            rhs=quarters[t // 4][:, ds((t % 4) * 2, 2), :],
                            start=(t == 0), stop=(t == NKC // 2 - 1),
                            perf_mode=PM.DoubleRow,
                        )
                    pps.append(ops)
                return pps

            def denom(b, quarters):
                # softmax denominator: fp8 DoubleRow ones-matmuls over
                # every quarter pair, one [1, QB] psum accumulation
                # group; the 64x lhsT values fold in the V' scale.
                # Purely exp-gated (no DVE dependency), so the next
                # block's S matmuls aren't stalled behind DVE folds.
                dn_ps = row_ps.tile([1, QB], F32, tag="row")
                for i in range(16):
                    nc.tensor.matmul(
                        out=dn_ps[:, :],
                        lhsT=ones_f8[:, :, 0:1],
                        rhs=quarters[i // 4][:, ds((i % 4) * 2, 2), :],
                        start=(i == 0), stop=(i == 15),
                        perf_mode=PM.DoubleRow,
                    )
                # 1/denom = exp(-ln(denom)) on ACT (same table set as Exp)
                lnrow = row_pool.tile([1, QB], F32, tag="lnrow")
                nc.scalar.activation(
                    out=lnrow[:, :], in_=dn_ps[:, :], func=AF.Ln
                )
                rcprow = row_pool.tile([1, QB], F32, tag="rcprow",
                                       name=f"rcprow{b}")
                nc.scalar.activation(
                    out=rcprow[:, :], in_=lnrow[:, :], func=AF.Exp, scale=-1.0
                )
                rcp_rep = scr_pool.tile([128, QB], F32, tag="rcprep",
                                        name=f"rcprep{b}")
                nc.gpsimd.partition_broadcast(rcp_rep[:, :], rcprow[:, :])
                return rcprow, rcp_rep

            def make_y(b, pps, rcp_rep):
                qsl = ds(b * QB, QB)
                y_sb = yt_pool.tile([128, 2, QB], F32R, tag="y", name=f"y{b}")
                for c in range(2):
                    ysc = scr_pool.tile([128, QB], F32, tag="scr")
                    nc.vector.tensor_mul(
                        out=ysc[:, :], in0=pps[c][:, :], in1=rcp_rep[:, :]
                    )
                    nc.vector.scalar_tensor_tensor(
                        out=y_sb[:, c, :],
                        in0=ysc[:, :],
                        scalar=pvec[:, ds(OBIAS + c, 1)],
                        in1=low_sb[:, c, qsl].bitcast(F32),
                        op0=OP.add, op1=OP.add,
                    )
                return y_sb

            def stats_ln(b, y_sb, rcprow):
                sy_ps = row_ps.tile([1, QB], F32, tag="row")
                for c in range(2):
                    nc.tensor.matmul(
                        out=sy_ps[:, :],
                        lhsT=ones128[:, :],
                        rhs=y_sb[:, c, :],
                        start=(c == 0), stop=(c == 1),
                    )
                murow = row_pool.tile([1, QB], F32, tag="murow")
                nc.vector.tensor_scalar_mul(
                    out=murow[:, :], in0=sy_ps[:, :], scalar1=1.0 / C
                )
                sy2_ps = row_ps.tile([1, QB], F32, tag="row")
                for c in range(2):
                    ysq = scr_pool.tile([128, QB], F32R, tag="ysq")
                    nc.vector.tensor_mul(
                        out=ysq[:, :],
                        in0=y_sb[:, c, :].bitcast(F32),
                        in1=y_sb[:, c, :].bitcast(F32),
                    )
                    nc.tensor.matmul(
                        out=sy2_ps[:, :],
                        lhsT=ones128[:, :],
                        rhs=ysq[:, :],
                        start=(c == 0), stop=(c == 1),
                    )
                # var = E[y^2] - mu^2 ; rstd = exp(-0.5 ln(var + eps))
                varrow = row_pool.tile([1, QB], F32, tag="varrow")
                nc.vector.tensor_scalar_mul(
                    out=varrow[:, :], in0=sy2_ps[:, :], scalar1=1.0 / C
                )
                mu2row = row_pool.tile([1, QB], F32, tag="mu2row")
                nc.vector.tensor_mul(
                    out=mu2row[:, :], in0=murow[:, :], in1=murow[:, :],
                )
                nc.vector.tensor_sub(
                    out=varrow[:, :], in0=varrow[:, :], in1=mu2row[:, :]
                )
                lnv = row_pool.tile([1, QB], F32, tag="lnv")
                nc.scalar.activation(
                    out=lnv[:, :], in_=varrow[:, :], func=AF.Ln, bias=epsb[:, :]
                )
                rstdrow = row_pool.tile([1, QB], F32, tag="rstdrow")
                nc.scalar.activation(
                    out=rstdrow[:, :], in_=lnv[:, :], func=AF.Exp, scale=-0.5
                )
                if dbg_d and b == NQB - 1:
                    nc.sync.dma_start(out=dbg_d["dbg_rcp"][:, :], in_=rcprow[:, :])
                    nc.sync.dma_start(out=dbg_d["dbg_mu"][:, :], in_=murow[:, :])
                    nc.sync.dma_start(out=dbg_d["dbg_var"][:, :],
                                      in_=varrow[:, :])
                    nc.sync.dma_start(out=dbg_d["dbg_rstd"][:, :],
                                      in_=rstdrow[:, :])
                mu_rep = scr_pool.tile([128, QB], F32, tag="murep")
                nc.gpsimd.partition_broadcast(mu_rep[:, :], murow[:, :])
                rs_rep = scr_pool.tile([128, QB], F32, tag="rsrep")
                nc.gpsimd.partition_broadcast(rs_rep[:, :], rstdrow[:, :])
                qsl = ds(b * QB, QB)
                osb = out_pool.tile([128, 2, QB], F32)
                for c in range(2):
                    yn = scr_pool.tile([128, QB], F32, tag="scr")
                    nc.vector.tensor_sub(
                        out=yn[:, :],
                        in0=y_sb[:, c, :].bitcast(F32),
                        in1=mu_rep[:, :],
                    )
                    nc.vector.tensor_mul(
                        out=yn[:, :], in0=yn[:, :], in1=rs_rep[:, :]
                    )
                    nc.vector.tensor_scalar(
                        out=osb[:, c, :], in0=yn[:, :],
                        scalar1=pvec[:, ds(LNG + c, 1)],
                        scalar2=pvec[:, ds(LNB + c, 1)],
                        op0=OP.mult, op1=OP.add,
                    )
                nc.sync.dma_start(
                    out=out_d[:, qsl].rearrange("(j p) q -> p j q", j=2),
                    in_=osb[:, :, :],
                )

            k_proj()
            q_proj()
            quarters0 = attention(0)
            v_proj()
            for b in range(NQB):
                quarters = quarters0 if b == 0 else attention(b)
                pps = pv(b, quarters)
                rcprow, rcp_rep = denom(b, quarters)
                y_b = make_y(b, pps, rcp_rep)
                stats_ln(b, y_b, rcprow)
                if dbg_d and b == NQB - 1:
                    nc.sync.dma_start(
                        out=dbg_d["dbg_pps"][:, 0, :], in_=pps[0][:, :]
                    )
                    nc.sync.dma_start(out=dbg_d["dbg_qt"][:, :, :],
                                      in_=qt_all[:, :, 3 * QB:4 * QB])
                    nc.sync.dma_start(
                        out=dbg_d["dbg_kt"][:, :, :], in_=kt_sb[0][:, :, :]
                    )
                    nc.sync.dma_start(
                        out=dbg_d["dbg_v"][:, :, :], in_=v_sb[0][:, :, :]
                    )
                    nc.sync.dma_start(
                        out=dbg_d["dbg_pt"][:, :, :], in_=quarters[3][:, :, :]
                    )

    # Force Exp and Ln to resolve to the one table set containing both
    # (the default chooser alternates exp_and_others <-> natural_log_exp,
    # paying a ~1.3us table load per switch, ~17 loads per kernel).
    import bass_rust as _br
    from concourse.hw_specs import get_activation_tables as _gat

    def _patched_act_loads():
        has_act = any(
            isinstance(i, mybir.InstActivation)
            for blk in nc.main_func.blocks for i in blk.instructions
        )
        if not has_act:
            return
        tables = []
        for name, fns in _gat(nc.m.arch).items():
            if name != "natural_log_exp_and_others":
                fns = fns - {AF.Exp, AF.Ln}
            tables.append((name, fns))
        _br.insert_act_table_loads(nc, tables)

    nc.insert_act_table_loads = _patched_act_loads
    nc.compile()
    return nc


def get_nc(dbg=False):
    key = "nc_dbg" if dbg else "nc"
    if key not in _CACHE:
        _CACHE[key] = _build_nc(dbg)
    return _CACHE[key]


def make_in_maps(low, high, q_w, q_b, k_w, k_b, v_w, v_b, o_w, o_b, ln_g, ln_b):
    import ml_dtypes
    f32 = lambda x: np.ascontiguousarray(np.asarray(x, np.float32))
    f8 = lambda x: np.ascontiguousarray(
        np.asarray(x, np.float32).astype(ml_dtypes.float8_e4m3)
    )
    low_r = np.asarray(low, np.float32).reshape(B, C, N)
    high_r = np.asarray(high, np.float32).reshape(B, C, N)
    # v-bias is exactly equivalent to an out-proj bias shift because the
    # softmax rows sum to one: attn @ (V + 1 vb^T) @ o_w^T = attn @ V @ o_w^T
    # + (o_w @ v_b)^T, so fold it on the host. The out-projection itself
    # folds into V: attn @ V @ o_w^T = attn @ (high_t @ (o_w @ v_w).T).
    o_w = np.asarray(o_w, np.float32)
    v_w = np.asarray(v_w, np.float32)
    ob_eff = np.asarray(o_b, np.float32) + o_w @ np.asarray(v_b, np.float32)
    w_vo = (o_w @ v_w) * SV
    pv_cols = []
    for v in [np.asarray(q_b, np.float32) * SQK,
              np.asarray(k_b, np.float32) * SQK, ob_eff, ln_g, ln_b]:
        pv_cols.append(np.asarray(v, np.float32).reshape(2, 128).T)
    shared = {
        "wq": f8(np.asarray(q_w, np.float32).T * SQK),
        "wk": f8(np.asarray(k_w, np.float32).T * SQK),
        "wvo": f8(w_vo.T),
        "pvec": f32(np.concatenate(pv_cols, axis=1)),
    }
    in_maps = []
    for i in range(8):
        bidx, h = i // 2, i % 2
        in_maps.append({
            "low": f32(low_r[bidx][:, h * NQ:(h + 1) * NQ]),
            "lowq": f8(low_r[bidx][:, h * NQ:(h + 1) * NQ]),
            "high": f8(high_r[bidx]),
            **shared,
        })
    return in_maps


def assemble(results):
    out = np.empty((B, C, N), np.float32)
    for i in range(8):
        bidx, h = i // 2, i % 2
        out[bidx][:, h * NQ:(h + 1) * NQ] = results[i]["out"]
    return out.reshape(B, C, 64, 64)


def kernel(**inputs) -> np.ndarray:
    nc = get_nc()
    in_maps = make_in_maps(**inputs)
    res = run_bass_kernel_spmd(nc, in_maps, core_ids=list(range(8)))
    return assemble(res.results)


if __name__ == "__main__":
    pass
